# revision 15
# baseline (speedup 1.0000x reference)
"""Multi-head attention (B=16, T=1024, D=768, H=12) on 8 TRN2 NeuronCores.

Strategy: pure data parallelism over the batch — each core computes full MHA
for 2 batch elements. No collectives.

Device kernel design (per core, bf16 compute / fp32 accumulate):
  - Host pre-transposes x to xT[b] = x[b].T ([D, T]) and pre-packs all weights
    in SBUF-ready layouts, cast to bf16.
  - Heads are processed in pairs (2 x HS = 128 = partition width).
  - qT/kT ([128, T], head pair stacked on partitions) come from
    matmul(lhsT=W_pair[dchunk, 128], rhs=xT[dchunk, T]) accumulated over D.
  - S^T[s, t] per head via row-tiled (tile_position) K=64 matmuls packing both
    heads of a pair into the 128-row PE array concurrently.
  - exp via ScalarE activation (scale=1/sqrt(HS) folded in, no max subtraction:
    |S|/8 <= ~3 for this data, exp is safe in fp32->bf16).
  - O^T = v_aug^T @ expS^T with v_aug = [v | ones]: row 64 of the PSUM result
    is the softmax denominator l[t] for free.
  - l -> 1/l via reciprocal_approx_fast (rows of several (h, th) batched on
    partitions via small DMA gathers), broadcast across partitions via a
    DRAM-bounce DMA, normalization as a single DVE multiply per (h, th).
  - y = O_all @ Wp + bp with lhsT = O_all^T (naturally produced above).
"""

import os
from contextlib import ExitStack

import numpy as np
import ml_dtypes

import concourse.bacc as bacc
import concourse.bass as bass
import concourse.mybir as mybir
import concourse.tile as tile
from concourse.bass_utils import run_bass_kernel_spmd

BF16 = ml_dtypes.bfloat16

# Full problem dims
B, T_FULL, D_FULL, H, HS = 16, 1024, 768, 12, 64
N_CORES = 8
NB = B // N_CORES  # batch elements per core


def build_mha_nc(nb, t, d, npair, trn_type="TRN2", variant="full"):
    """Build the Bass program for `nb` batch elements, seq len `t`, model dim
    `d`, `npair` head pairs (each pair = 128 partition lanes)."""
    P = 128
    KC = d // P              # contraction chunks over model dim
    SC = t // P              # s (key position) chunks
    NTH = max(1, t // 512)   # output-column groups for S/O matmuls
    TW = t // NTH            # width of each group (<= 512)
    TC = t // P              # t row chunks for v/y
    D2 = d // 2              # y-proj free-dim split (<= 512 fp32 psum)
    dpair = 2 * HS           # 128
    scale = 1.0 / np.sqrt(HS)

    f32 = mybir.dt.float32
    bf16 = mybir.dt.bfloat16
    AF = mybir.ActivationFunctionType

    nc = bacc.Bacc(trn_type, target_bir_lowering=False, debug=False)

    xt_d = nc.dram_tensor("xt", [nb, d, t], bf16, kind="ExternalInput").ap()
    wq_d = nc.dram_tensor("wq", [P, npair, KC, dpair], bf16, kind="ExternalInput").ap()
    wk_d = nc.dram_tensor("wk", [P, npair, KC, dpair], bf16, kind="ExternalInput").ap()
    wv_d = nc.dram_tensor("wv", [P, KC, npair * dpair], bf16, kind="ExternalInput").ap()
    wp_d = nc.dram_tensor("wp", [P, KC, d], bf16, kind="ExternalInput").ap()
    bqk_d = nc.dram_tensor("bqk", [P, npair, 2], f32, kind="ExternalInput").ap()
    bv_d = nc.dram_tensor("bv", [P, npair, dpair], bf16, kind="ExternalInput").ap()
    bp_d = nc.dram_tensor("bp", [P, d], f32, kind="ExternalInput").ap()
    y_d = nc.dram_tensor("y", [nb, t, d], f32, kind="ExternalOutput").ap()

    with TileOrExit(nc) as (tc, ctx):
        # ---- persistent weights (one bufs=1 pool; each tag allocated once) ----
        p_w = ctx.enter_context(tc.tile_pool(name="p_w", bufs=1))
        wq_sb = p_w.tile([P, npair, KC, dpair], bf16, tag="wq", name="wq_sb")
        wk_sb = p_w.tile([P, npair, KC, dpair], bf16, tag="wk", name="wk_sb")
        wv_sb = p_w.tile([P, KC, npair * dpair], bf16, tag="wv", name="wv_sb")
        wp_sb = p_w.tile([P, KC, d], bf16, tag="wp", name="wp_sb")
        bqk_sb = p_w.tile([P, npair, 2], f32, tag="bqk", name="bqk_sb")
        bv_sb = p_w.tile([P, npair, dpair], bf16, tag="bv", name="bv_sb")
        bp_sb = p_w.tile([P, d], f32, tag="bp", name="bp_sb")
        # weight loads ride the gpsimd DMA queue so the sync queue is free for
        # xt (first compute dependency); split by chunk for fine-grained deps
        for c in range(KC):
            nc.gpsimd.dma_start(wv_sb[:, c], wv_d[:, c])
        nc.gpsimd.dma_start(bv_sb[:], bv_d)
        for pr in range(npair):
            nc.gpsimd.dma_start(wq_sb[:, pr], wq_d[:, pr])
            nc.gpsimd.dma_start(wk_sb[:, pr], wk_d[:, pr])
        nc.gpsimd.dma_start(bqk_sb[:], bqk_d)
        nc.gpsimd.dma_start(wp_sb[:], wp_d)
        nc.gpsimd.dma_start(bp_sb[:], bp_d)

        # ---- pools ----
        p_xt = ctx.enter_context(tc.tile_pool(name="p_xt", bufs=2))
        p_vall = ctx.enter_context(tc.tile_pool(name="p_vall", bufs=2))
        p_qk = ctx.enter_context(tc.tile_pool(name="p_qk", bufs=2))
        p_es = ctx.enter_context(tc.tile_pool(name="p_es", bufs=2))
        p_oall = ctx.enter_context(tc.tile_pool(name="p_oall", bufs=1))
        p_norm = ctx.enter_context(tc.tile_pool(name="p_norm", bufs=2))
        p_y = ctx.enter_context(tc.tile_pool(name="p_y", bufs=2))
        p_dram = ctx.enter_context(tc.tile_pool(name="p_dram", bufs=2, space="DRAM"))
        ps_s = ctx.enter_context(tc.tile_pool(name="ps_s", bufs=2, space="PSUM"))
        ps_o = ctx.enter_context(tc.tile_pool(name="ps_o", bufs=2, space="PSUM"))
        ps_m = ctx.enter_context(tc.tile_pool(name="ps_m", bufs=2, space="PSUM"))

        for b in range(nb):
            xt = p_xt.tile([P, KC, t], bf16, tag="xt", name="xt_sb")
            xt_src = xt_d[b].rearrange("(c p) t -> p c t", p=P)
            for c in range(KC):
                nc.sync.dma_start(xt[:, c], xt_src[:, c])

            # ---- v projection: v_all[:, sc, pair, 0:65]   = [v_h0 | ones]
            #                    v_all[:, sc, pair, 65:130] = [v_h1 | ones]
            v_all = p_vall.tile([P, SC, npair, 130], bf16, tag="vall", name="v_all")
            ones_view = v_all.rearrange("p s r (h x) -> p s r h x", h=2)
            nc.gpsimd.memset(ones_view[:, :, :, :, 64:65], 1.0)
            nhalf = (npair + 2) // 3  # groups of <=3 pairs per psum tile
            for tci in range(TC):
                gns = [min(3, npair - 3 * g) for g in range(nhalf)]
                psvs = [ps_m.tile([P, 3 * dpair], f32, tag="m", name="psv") for _ in range(nhalf)]
                for c in range(KC):
                    for g in range(nhalf):
                        nc.tensor.matmul(
                            psvs[g][:, : gns[g] * dpair],
                            lhsT=xt[:, c, tci * P : (tci + 1) * P],
                            rhs=wv_sb[:, c, 3 * g * dpair : (3 * g + gns[g]) * dpair],
                            start=(c == 0),
                            stop=(c == KC - 1),
                        )
                for g in range(nhalf):
                    glo, gn = 3 * g, gns[g]
                    dst = v_all[:, tci, glo : glo + gn, :].rearrange(
                        "p r (h x) -> p r h x", h=2
                    )[:, :, :, 0:64]
                    src = psvs[g][:, : gn * dpair].rearrange("p (r h e) -> p r h e", r=gn, h=2)
                    bias = bv_sb[:, glo : glo + gn, :].rearrange("p r (h e) -> p r h e", h=2)
                    nc.vector.tensor_add(out=dst, in0=src, in1=bias)

            o_allT = p_oall.tile([P, npair, t], bf16, tag="oall", name="o_allT")

            def qk_quanta(pr):
                """Emit pair pr's q/k projections as a list of closures, one
                PE matmul (or trailing DVE bias-copy) each, for interleaving
                into the previous pair's S/O tick stream."""
                qT = p_qk.tile([P, t], bf16, tag="qT", name="qT")
                kT = p_qk.tile([P, t], bf16, tag="kT", name="kT")
                quanta = []
                psqs = {}

                def alloc(bj):
                    psqs[bj] = [ps_m.tile([P, TW], f32, tag="m", name="psq") for _ in range(NTH)]

                def mm(w_sb, bj, c, th):
                    def _go():
                        nc.tensor.matmul(
                            psqs[bj][th][:],
                            lhsT=w_sb[:, pr, c, :],
                            rhs=xt[:, c, th * TW : (th + 1) * TW],
                            start=(c == 0),
                            stop=(c == KC - 1),
                        )
                    return _go

                def bias(bj, dstT, th):
                    def _go():
                        nc.vector.tensor_scalar_add(
                            out=dstT[:, th * TW : (th + 1) * TW],
                            in0=psqs[bj][th][:],
                            scalar1=bqk_sb[:, pr, bj : bj + 1],
                        )
                    return _go

                for w_sb, bj, dstT in ((wq_sb, 0, qT), (wk_sb, 1, kT)):
                    quanta.append(lambda bj=bj: alloc(bj))
                    for c in range(KC):
                        for th in range(NTH):
                            quanta.append(mm(w_sb, bj, c, th))
                    for th in range(NTH):
                        quanta.append(bias(bj, dstT, th))
                return qT, kT, quanta

            def run_all(quanta):
                for q in quanta:
                    q()

            qT, kT, quanta = qk_quanta(0)
            run_all(quanta)
            for pr in range(npair):
                if pr + 1 < npair:
                    nqT, nkT, quanta = qk_quanta(pr + 1)
                else:
                    nqT = nkT = None
                    quanta = []
                qi = [0]
                n_ticks = NTH * (SC + 1)
                per_tick = -(-len(quanta) // max(1, n_ticks - 4))

                def drip():
                    for _ in range(per_tick):
                        if qi[0] < len(quanta):
                            quanta[qi[0]]()
                            qi[0] += 1

                # ---- fused S -> exp -> O pipeline per t-half, with one-stage
                # skew (S(sc+1) emitted before O(sc)) and qk(p+1) dripped in
                for th in range(NTH):
                    es = p_es.tile([P, SC, 2, TW], bf16, tag="es", name="es")
                    psos = [ps_o.tile([65, TW], f32, tag="o", name="pso") for _ in range(2)]
                    for sc in range(SC + 1):
                        if sc < SC:
                            ps = ps_s.tile([P, 2, TW], f32, tag="s", name="ps_s")
                            nc.tensor.matmul(
                                ps[:, 0, :],
                                lhsT=kT[0:64, sc * P : (sc + 1) * P],
                                rhs=qT[0:64, th * TW : (th + 1) * TW],
                                start=True,
                                stop=True,
                            )
                            nc.tensor.matmul(
                                ps[:, 1, :],
                                lhsT=kT[64:128, sc * P : (sc + 1) * P],
                                rhs=qT[64:128, th * TW : (th + 1) * TW],
                                start=True,
                                stop=True,
                                tile_position=None if "notile" in variant else (64, 0),
                            )
                            nc.scalar.activation(
                                out=es[:, sc, :, :], in_=ps[:], func=AF.Exp, scale=scale
                            )
                        drip()
                        if sc >= 1:
                            so = sc - 1
                            for h in range(2):
                                nc.tensor.matmul(
                                    psos[h][:],
                                    lhsT=v_all[:, so, pr, 65 * h : 65 * h + 65],
                                    rhs=es[:, so, h, :],
                                    start=(so == 0),
                                    stop=(so == SC - 1),
                                )
                    # gather the two l rows, invert, broadcast, normalize
                    l_sb = p_norm.tile([65, 2, TW], f32, tag="l", name="l_sb")
                    for h in range(2):
                        nc.vector.tensor_copy(out=l_sb[64:65, h, :], in_=psos[h][64:65, :])
                    lg = p_norm.tile([2, TW], f32, tag="lg", name="lg")
                    for h in range(2):
                        nc.sync.dma_start(out=lg[h : h + 1, :], in_=l_sb[64:65, h, :])
                    lginv = p_norm.tile([2, TW], f32, tag="lginv", name="lginv")
                    if "norecip" in variant:
                        nc.vector.tensor_copy(out=lginv[:], in_=lg[:])
                    else:
                        nc.vector.reciprocal_approx_fast(out=lginv[:], in_=lg[:])
                    lgbf = p_norm.tile([2, TW], bf16, tag="lgbf", name="lgbf")
                    nc.vector.tensor_copy(out=lgbf[:], in_=lginv[:])
                    dr = p_dram.tile([2, TW], bf16, tag="dr", name="dr")
                    nc.sync.dma_start(out=dr[:], in_=lgbf[:])
                    linv = p_norm.tile([64, 2, TW], bf16, tag="linv", name="linv")
                    if "nobcast" in variant:
                        nc.vector.memset(linv[:], 1.0)
                    else:
                        nc.sync.dma_start(
                            out=linv[:],
                            in_=dr[:].rearrange("(o a) b -> o a b", o=1).to_broadcast([64, 2, TW]),
                        )
                    for h in range(2):
                        if h == 0:
                            nc.vector.tensor_mul(
                                out=o_allT[0:64, pr, th * TW : (th + 1) * TW],
                                in0=psos[h][0:64, :],
                                in1=linv[:, h, :],
                            )
                        else:
                            ot = p_norm.tile([64, TW], bf16, tag="ot", name="ot")
                            nc.vector.tensor_mul(out=ot[:], in0=psos[h][0:64, :], in1=linv[:, h, :])
                            nc.sync.dma_start(
                                out=o_allT[64:128, pr, th * TW : (th + 1) * TW], in_=ot[:]
                            )
                run_all(quanta[qi[0] :])
                if nqT is not None:
                    qT, kT = nqT, nkT

            # ---- output projection y = O_all @ Wp + bp
            for tci in range(TC):
                psy = [ps_m.tile([P, D2], f32, tag="m", name="psy") for _ in range(2)]
                for c in range(KC):
                    for j in range(2):
                        nc.tensor.matmul(
                            psy[j][:],
                            lhsT=o_allT[:, c, tci * P : (tci + 1) * P],
                            rhs=wp_sb[:, c, j * D2 : (j + 1) * D2],
                            start=(c == 0),
                            stop=(c == KC - 1),
                        )
                y_sb = p_y.tile([P, d], f32, tag="y", name="y_sb")
                for j in range(2):
                    nc.vector.tensor_add(
                        out=y_sb[:, j * D2 : (j + 1) * D2],
                        in0=psy[j][:],
                        in1=bp_sb[:, j * D2 : (j + 1) * D2],
                    )
                nc.sync.dma_start(out=y_d[b, tci * P : (tci + 1) * P, :], in_=y_sb[:])

    nc.compile()
    return nc


class TileOrExit:
    """Combined TileContext + ExitStack context manager."""

    def __init__(self, nc):
        self.nc = nc
        self.ctx = ExitStack()
        self.tc = tile.TileContext(nc)

    def __enter__(self):
        self.ctx.__enter__()
        self.tc.__enter__()
        return self.tc, self.ctx

    def __exit__(self, *a):
        # close pools before TileContext exits scheduling
        self.ctx.__exit__(*a)
        return self.tc.__exit__(*a)


def prep_inputs(x, Wq, bq, Wk, bk, Wv, bv, Wp, bp, nb, npair):
    """Host-side packing into the DRAM layouts the device kernel expects.

    Returns (shared weight map, list of per-core input maps)."""
    P = 128
    t = x.shape[1]
    d = x.shape[2]
    KC = d // P
    dpair = 2 * HS

    def to_bf(a):
        return np.ascontiguousarray(a).astype(BF16)

    # x^T per batch element
    xt = np.ascontiguousarray(x.transpose(0, 2, 1)).astype(BF16)  # [B, d, t]

    # wq/wk: [P, pair, c, 128] with cols 0:64 = head 2p, 64:128 = head 2p+1
    def pack_qk(W):
        # W: [H, d, HS] -> [pair, 2, KC, P, HS] -> [P, pair, KC, 2*HS]
        w = W.reshape(npair, 2, KC, P, HS)
        w = w.transpose(3, 0, 2, 1, 4).reshape(P, npair, KC, dpair)
        return to_bf(w)

    wq = pack_qk(Wq)
    wk = pack_qk(Wk)
    wv = pack_qk(Wv).transpose(0, 2, 1, 3).reshape(P, KC, npair * dpair)
    wv = np.ascontiguousarray(wv)  # [P, c, pair*128]
    # wp: [P, c, d]
    wp = to_bf(Wp.reshape(KC, P, d).transpose(1, 0, 2))
    # bqk: [P, pair, 2] fp32: partition = pair-stacked head dims
    bqk = np.stack(
        [bq.reshape(npair, dpair), bk.reshape(npair, dpair)], axis=-1
    )  # [pair, 128, 2]
    bqk = np.ascontiguousarray(bqk.transpose(1, 0, 2)).astype(np.float32)  # [P, pair, 2]
    # bv broadcast along t partitions: [P, pair, 128]
    bv_bc = np.broadcast_to(bv.reshape(1, npair, dpair), (P, npair, dpair))
    bv_bc = to_bf(bv_bc)
    # bp broadcast: [P, d] fp32
    bp_bc = np.ascontiguousarray(np.broadcast_to(bp.reshape(1, d), (P, d))).astype(
        np.float32
    )

    weights = {
        "wq": wq,
        "wk": wk,
        "wv": wv,
        "wp": wp,
        "bqk": bqk,
        "bv": bv_bc,
        "bp": bp_bc,
    }
    n_cores = x.shape[0] // nb
    in_maps = []
    for i in range(n_cores):
        m = dict(weights)
        m["xt"] = np.ascontiguousarray(xt[i * nb : (i + 1) * nb])
        in_maps.append(m)
    return in_maps


_NC_CACHE = {}
LAST_RESULT = {}


def kernel(x, Wq, bq, Wk, bk, Wv, bv, Wp, bp, _trace=False):
    x = np.asarray(x, dtype=np.float32)
    Wq, bq = np.asarray(Wq, np.float32), np.asarray(bq, np.float32)
    Wk, bk = np.asarray(Wk, np.float32), np.asarray(bk, np.float32)
    Wv, bv = np.asarray(Wv, np.float32), np.asarray(bv, np.float32)
    Wp, bp = np.asarray(Wp, np.float32), np.asarray(bp, np.float32)

    npair = H // 2
    key = ("full", NB, T_FULL, D_FULL, npair)
    if key not in _NC_CACHE:
        _NC_CACHE[key] = build_mha_nc(NB, T_FULL, D_FULL, npair)
    nc = _NC_CACHE[key]

    in_maps = prep_inputs(x, Wq, bq, Wk, bk, Wv, bv, Wp, bp, NB, npair)
    res = run_bass_kernel_spmd(
        nc, in_maps, core_ids=list(range(N_CORES)), trace=_trace
    )
    LAST_RESULT["exec_time_ns"] = res.exec_time_ns
    LAST_RESULT["res"] = res
    outs = [res.results[i]["y"] for i in range(N_CORES)]
    return np.concatenate(outs, axis=0).astype(np.float32)


# revision 16
# speedup vs baseline: 1.0500x; 1.0500x over previous
"""Multi-head attention (B=16, T=1024, D=768, H=12) on 8 TRN2 NeuronCores.

Strategy: pure data parallelism over the batch — each core computes full MHA
for 2 batch elements. No collectives.

Device kernel design (per core, bf16 compute / fp32 accumulate):
  - Host pre-transposes x to xT[b] = x[b].T ([D, T]) and pre-packs all weights
    in SBUF-ready layouts, cast to bf16.
  - Heads are processed in pairs (2 x HS = 128 = partition width).
  - qT/kT ([128, T], head pair stacked on partitions) come from
    matmul(lhsT=W_pair[dchunk, 128], rhs=xT[dchunk, T]) accumulated over D.
  - S^T[s, t] per head via row-tiled (tile_position) K=64 matmuls packing both
    heads of a pair into the 128-row PE array concurrently.
  - exp via ScalarE activation (scale=1/sqrt(HS) folded in, no max subtraction:
    |S|/8 <= ~3 for this data, exp is safe in fp32->bf16).
  - O^T = v_aug^T @ expS^T with v_aug = [v | ones]: row 64 of the PSUM result
    is the softmax denominator l[t] for free.
  - l -> 1/l via reciprocal_approx_fast (rows of several (h, th) batched on
    partitions via small DMA gathers), broadcast across partitions via a
    DRAM-bounce DMA, normalization as a single DVE multiply per (h, th).
  - y = O_all @ Wp + bp with lhsT = O_all^T (naturally produced above).
"""

import os
from contextlib import ExitStack

import numpy as np
import ml_dtypes

import concourse.bacc as bacc
import concourse.bass as bass
import concourse.mybir as mybir
import concourse.tile as tile
from concourse.bass_utils import run_bass_kernel_spmd

BF16 = ml_dtypes.bfloat16

# Full problem dims
B, T_FULL, D_FULL, H, HS = 16, 1024, 768, 12, 64
N_CORES = 8
NB = B // N_CORES  # batch elements per core


def build_mha_nc(nb, t, d, npair, trn_type="TRN2", variant="full"):
    """Build the Bass program for `nb` batch elements, seq len `t`, model dim
    `d`, `npair` head pairs (each pair = 128 partition lanes)."""
    P = 128
    KC = d // P              # contraction chunks over model dim
    SC = t // P              # s (key position) chunks
    NTH = max(1, t // 512)   # output-column groups for S/O matmuls
    TW = t // NTH            # width of each group (<= 512)
    TC = t // P              # t row chunks for v/y
    D2 = d // 2              # y-proj free-dim split (<= 512 fp32 psum)
    dpair = 2 * HS           # 128
    scale = 1.0 / np.sqrt(HS)

    f32 = mybir.dt.float32
    bf16 = mybir.dt.bfloat16
    AF = mybir.ActivationFunctionType

    nc = bacc.Bacc(trn_type, target_bir_lowering=False, debug=False)

    xt_d = nc.dram_tensor("xt", [nb, d, t], bf16, kind="ExternalInput").ap()
    wq_d = nc.dram_tensor("wq", [P, npair, KC, dpair], bf16, kind="ExternalInput").ap()
    wk_d = nc.dram_tensor("wk", [P, npair, KC, dpair], bf16, kind="ExternalInput").ap()
    wv_d = nc.dram_tensor("wv", [P, KC, npair * dpair], bf16, kind="ExternalInput").ap()
    wp_d = nc.dram_tensor("wp", [P, KC, d], bf16, kind="ExternalInput").ap()
    bqk_d = nc.dram_tensor("bqk", [P, npair, 2], f32, kind="ExternalInput").ap()
    bv_d = nc.dram_tensor("bv", [P, npair, dpair], bf16, kind="ExternalInput").ap()
    bp_d = nc.dram_tensor("bp", [P, d], f32, kind="ExternalInput").ap()
    y_d = nc.dram_tensor("y", [nb, t, d], f32, kind="ExternalOutput").ap()

    with TileOrExit(nc) as (tc, ctx):
        # ---- persistent weights (one bufs=1 pool; each tag allocated once) ----
        p_w = ctx.enter_context(tc.tile_pool(name="p_w", bufs=1))
        wq_sb = p_w.tile([P, npair, KC, dpair], bf16, tag="wq", name="wq_sb")
        wk_sb = p_w.tile([P, npair, KC, dpair], bf16, tag="wk", name="wk_sb")
        wv_sb = p_w.tile([P, KC, npair * dpair], bf16, tag="wv", name="wv_sb")
        wp_sb = p_w.tile([P, KC, d], bf16, tag="wp", name="wp_sb")
        bqk_sb = p_w.tile([P, npair, 2], f32, tag="bqk", name="bqk_sb")
        bv_sb = p_w.tile([P, npair, dpair], bf16, tag="bv", name="bv_sb")
        bp_sb = p_w.tile([P, d], f32, tag="bp", name="bp_sb")
        # weight loads ride the gpsimd DMA queue so the sync queue is free for
        # xt (first compute dependency); split by chunk for fine-grained deps
        for c in range(KC):
            nc.gpsimd.dma_start(wv_sb[:, c], wv_d[:, c])
        nc.gpsimd.dma_start(bv_sb[:], bv_d)
        for pr in range(npair):
            nc.gpsimd.dma_start(wq_sb[:, pr], wq_d[:, pr])
            nc.gpsimd.dma_start(wk_sb[:, pr], wk_d[:, pr])
        nc.gpsimd.dma_start(bqk_sb[:], bqk_d)
        nc.gpsimd.dma_start(wp_sb[:], wp_d)
        nc.gpsimd.dma_start(bp_sb[:], bp_d)

        # ---- pools ----
        p_xt = ctx.enter_context(tc.tile_pool(name="p_xt", bufs=2))
        p_vall = ctx.enter_context(tc.tile_pool(name="p_vall", bufs=2))
        p_qk = ctx.enter_context(tc.tile_pool(name="p_qk", bufs=2))
        p_es = ctx.enter_context(tc.tile_pool(name="p_es", bufs=2))
        p_oall = ctx.enter_context(tc.tile_pool(name="p_oall", bufs=1))
        p_norm = ctx.enter_context(tc.tile_pool(name="p_norm", bufs=2))
        p_y = ctx.enter_context(tc.tile_pool(name="p_y", bufs=2))
        p_dram = ctx.enter_context(tc.tile_pool(name="p_dram", bufs=2, space="DRAM"))
        ps_s = ctx.enter_context(tc.tile_pool(name="ps_s", bufs=2, space="PSUM"))
        ps_o = ctx.enter_context(tc.tile_pool(name="ps_o", bufs=2, space="PSUM"))
        ps_m = ctx.enter_context(tc.tile_pool(name="ps_m", bufs=2, space="PSUM"))

        for b in range(nb):
            xt = p_xt.tile([P, KC, t], bf16, tag="xt", name="xt_sb")
            xt_src = xt_d[b].rearrange("(c p) t -> p c t", p=P)
            for c in range(KC):
                nc.sync.dma_start(xt[:, c], xt_src[:, c])

            # ---- v projection: v_all[:, sc, pair, 0:65]   = [v_h0 | ones]
            #                    v_all[:, sc, pair, 65:130] = [v_h1 | ones]
            v_all = p_vall.tile([P, SC, npair, 130], bf16, tag="vall", name="v_all")
            ones_view = v_all.rearrange("p s r (h x) -> p s r h x", h=2)
            nc.gpsimd.memset(ones_view[:, :, :, :, 64:65], 1.0)
            nhalf = (npair + 2) // 3  # groups of <=3 pairs per psum tile
            for tci in range(TC):
                gns = [min(3, npair - 3 * g) for g in range(nhalf)]
                psvs = [ps_m.tile([P, 3 * dpair], f32, tag="m", name="psv") for _ in range(nhalf)]
                for c in range(KC):
                    for g in range(nhalf):
                        nc.tensor.matmul(
                            psvs[g][:, : gns[g] * dpair],
                            lhsT=xt[:, c, tci * P : (tci + 1) * P],
                            rhs=wv_sb[:, c, 3 * g * dpair : (3 * g + gns[g]) * dpair],
                            start=(c == 0),
                            stop=(c == KC - 1),
                        )
                for g in range(nhalf):
                    glo, gn = 3 * g, gns[g]
                    dst = v_all[:, tci, glo : glo + gn, :].rearrange(
                        "p r (h x) -> p r h x", h=2
                    )[:, :, :, 0:64]
                    src = psvs[g][:, : gn * dpair].rearrange("p (r h e) -> p r h e", r=gn, h=2)
                    bias = bv_sb[:, glo : glo + gn, :].rearrange("p r (h e) -> p r h e", h=2)
                    nc.vector.tensor_add(out=dst, in0=src, in1=bias)

            o_allT = p_oall.tile([P, npair, t], bf16, tag="oall", name="o_allT")

            for pr in range(npair):
                # ---- q/k head-pair projections -> qT/kT [128, t] bf16
                qT = p_qk.tile([P, t], bf16, tag="qT", name="qT")
                kT = p_qk.tile([P, t], bf16, tag="kT", name="kT")
                for w_sb, bj, dstT in ((wq_sb, 0, qT), (wk_sb, 1, kT)):
                    psqs = [ps_m.tile([P, TW], f32, tag="m", name="psq") for _ in range(NTH)]
                    for c in range(KC):
                        for th in range(NTH):
                            nc.tensor.matmul(
                                psqs[th][:],
                                lhsT=w_sb[:, pr, c, :],
                                rhs=xt[:, c, th * TW : (th + 1) * TW],
                                start=(c == 0),
                                stop=(c == KC - 1),
                            )
                    for th in range(NTH):
                        nc.vector.tensor_scalar_add(
                            out=dstT[:, th * TW : (th + 1) * TW],
                            in0=psqs[th][:],
                            scalar1=bqk_sb[:, pr, bj : bj + 1],
                        )

                # ---- fused S -> exp -> O pipeline per t-half, with one-stage
                # skew (S(sc+1) emitted before O(sc)) and qk(p+1) dripped in
                for th in range(NTH):
                    es = p_es.tile([P, SC, 2, TW], bf16, tag="es", name="es")
                    psos = [ps_o.tile([65, TW], f32, tag="o", name="pso") for _ in range(2)]
                    for sc in range(SC + 1):
                        if sc < SC:
                            ps = ps_s.tile([P, 2, TW], f32, tag="s", name="ps_s")
                            nc.tensor.matmul(
                                ps[:, 0, :],
                                lhsT=kT[0:64, sc * P : (sc + 1) * P],
                                rhs=qT[0:64, th * TW : (th + 1) * TW],
                                start=True,
                                stop=True,
                            )
                            nc.tensor.matmul(
                                ps[:, 1, :],
                                lhsT=kT[64:128, sc * P : (sc + 1) * P],
                                rhs=qT[64:128, th * TW : (th + 1) * TW],
                                start=True,
                                stop=True,
                                tile_position=None if "notile" in variant else (64, 0),
                            )
                            nc.scalar.activation(
                                out=es[:, sc, :, :], in_=ps[:], func=AF.Exp, scale=scale
                            )
                        if sc >= 1:
                            so = sc - 1
                            for h in range(2):
                                nc.tensor.matmul(
                                    psos[h][:],
                                    lhsT=v_all[:, so, pr, 65 * h : 65 * h + 65],
                                    rhs=es[:, so, h, :],
                                    start=(so == 0),
                                    stop=(so == SC - 1),
                                )
                    # gather the two l rows, invert, broadcast, normalize
                    l_sb = p_norm.tile([65, 2, TW], f32, tag="l", name="l_sb")
                    for h in range(2):
                        nc.vector.tensor_copy(out=l_sb[64:65, h, :], in_=psos[h][64:65, :])
                    lg = p_norm.tile([2, TW], f32, tag="lg", name="lg")
                    for h in range(2):
                        nc.sync.dma_start(out=lg[h : h + 1, :], in_=l_sb[64:65, h, :])
                    lginv = p_norm.tile([2, TW], f32, tag="lginv", name="lginv")
                    if "norecip" in variant:
                        nc.vector.tensor_copy(out=lginv[:], in_=lg[:])
                    else:
                        nc.vector.reciprocal_approx_fast(out=lginv[:], in_=lg[:])
                    lgbf = p_norm.tile([2, TW], bf16, tag="lgbf", name="lgbf")
                    nc.vector.tensor_copy(out=lgbf[:], in_=lginv[:])
                    dr = p_dram.tile([2, TW], bf16, tag="dr", name="dr")
                    nc.sync.dma_start(out=dr[:], in_=lgbf[:])
                    linv = p_norm.tile([64, 2, TW], bf16, tag="linv", name="linv")
                    if "nobcast" in variant:
                        nc.vector.memset(linv[:], 1.0)
                    else:
                        nc.sync.dma_start(
                            out=linv[:],
                            in_=dr[:].rearrange("(o a) b -> o a b", o=1).to_broadcast([64, 2, TW]),
                        )
                    for h in range(2):
                        if h == 0:
                            nc.vector.tensor_mul(
                                out=o_allT[0:64, pr, th * TW : (th + 1) * TW],
                                in0=psos[h][0:64, :],
                                in1=linv[:, h, :],
                            )
                        else:
                            ot = p_norm.tile([64, TW], bf16, tag="ot", name="ot")
                            nc.vector.tensor_mul(out=ot[:], in0=psos[h][0:64, :], in1=linv[:, h, :])
                            nc.sync.dma_start(
                                out=o_allT[64:128, pr, th * TW : (th + 1) * TW], in_=ot[:]
                            )
            # ---- output projection y = O_all @ Wp + bp
            for tci in range(TC):
                psy = [ps_o.tile([P, D2], f32, tag="o", name="psy") for _ in range(2)]
                for c in range(KC):
                    for j in range(2):
                        nc.tensor.matmul(
                            psy[j][:],
                            lhsT=o_allT[:, c, tci * P : (tci + 1) * P],
                            rhs=wp_sb[:, c, j * D2 : (j + 1) * D2],
                            start=(c == 0),
                            stop=(c == KC - 1),
                        )
                y_sb = p_y.tile([P, d], f32, tag="y", name="y_sb")
                for j in range(2):
                    nc.vector.tensor_add(
                        out=y_sb[:, j * D2 : (j + 1) * D2],
                        in0=psy[j][:],
                        in1=bp_sb[:, j * D2 : (j + 1) * D2],
                    )
                nc.sync.dma_start(out=y_d[b, tci * P : (tci + 1) * P, :], in_=y_sb[:])

    nc.compile()
    return nc


class TileOrExit:
    """Combined TileContext + ExitStack context manager."""

    def __init__(self, nc):
        self.nc = nc
        self.ctx = ExitStack()
        self.tc = tile.TileContext(nc)

    def __enter__(self):
        self.ctx.__enter__()
        self.tc.__enter__()
        return self.tc, self.ctx

    def __exit__(self, *a):
        # close pools before TileContext exits scheduling
        self.ctx.__exit__(*a)
        return self.tc.__exit__(*a)


def prep_inputs(x, Wq, bq, Wk, bk, Wv, bv, Wp, bp, nb, npair):
    """Host-side packing into the DRAM layouts the device kernel expects.

    Returns (shared weight map, list of per-core input maps)."""
    P = 128
    t = x.shape[1]
    d = x.shape[2]
    KC = d // P
    dpair = 2 * HS

    def to_bf(a):
        return np.ascontiguousarray(a).astype(BF16)

    # x^T per batch element
    xt = np.ascontiguousarray(x.transpose(0, 2, 1)).astype(BF16)  # [B, d, t]

    # wq/wk: [P, pair, c, 128] with cols 0:64 = head 2p, 64:128 = head 2p+1
    def pack_qk(W):
        # W: [H, d, HS] -> [pair, 2, KC, P, HS] -> [P, pair, KC, 2*HS]
        w = W.reshape(npair, 2, KC, P, HS)
        w = w.transpose(3, 0, 2, 1, 4).reshape(P, npair, KC, dpair)
        return to_bf(w)

    wq = pack_qk(Wq)
    wk = pack_qk(Wk)
    wv = pack_qk(Wv).transpose(0, 2, 1, 3).reshape(P, KC, npair * dpair)
    wv = np.ascontiguousarray(wv)  # [P, c, pair*128]
    # wp: [P, c, d]
    wp = to_bf(Wp.reshape(KC, P, d).transpose(1, 0, 2))
    # bqk: [P, pair, 2] fp32: partition = pair-stacked head dims
    bqk = np.stack(
        [bq.reshape(npair, dpair), bk.reshape(npair, dpair)], axis=-1
    )  # [pair, 128, 2]
    bqk = np.ascontiguousarray(bqk.transpose(1, 0, 2)).astype(np.float32)  # [P, pair, 2]
    # bv broadcast along t partitions: [P, pair, 128]
    bv_bc = np.broadcast_to(bv.reshape(1, npair, dpair), (P, npair, dpair))
    bv_bc = to_bf(bv_bc)
    # bp broadcast: [P, d] fp32
    bp_bc = np.ascontiguousarray(np.broadcast_to(bp.reshape(1, d), (P, d))).astype(
        np.float32
    )

    weights = {
        "wq": wq,
        "wk": wk,
        "wv": wv,
        "wp": wp,
        "bqk": bqk,
        "bv": bv_bc,
        "bp": bp_bc,
    }
    n_cores = x.shape[0] // nb
    in_maps = []
    for i in range(n_cores):
        m = dict(weights)
        m["xt"] = np.ascontiguousarray(xt[i * nb : (i + 1) * nb])
        in_maps.append(m)
    return in_maps


_NC_CACHE = {}
LAST_RESULT = {}


def kernel(x, Wq, bq, Wk, bk, Wv, bv, Wp, bp, _trace=False):
    x = np.asarray(x, dtype=np.float32)
    Wq, bq = np.asarray(Wq, np.float32), np.asarray(bq, np.float32)
    Wk, bk = np.asarray(Wk, np.float32), np.asarray(bk, np.float32)
    Wv, bv = np.asarray(Wv, np.float32), np.asarray(bv, np.float32)
    Wp, bp = np.asarray(Wp, np.float32), np.asarray(bp, np.float32)

    npair = H // 2
    key = ("full", NB, T_FULL, D_FULL, npair)
    if key not in _NC_CACHE:
        _NC_CACHE[key] = build_mha_nc(NB, T_FULL, D_FULL, npair)
    nc = _NC_CACHE[key]

    in_maps = prep_inputs(x, Wq, bq, Wk, bk, Wv, bv, Wp, bp, NB, npair)
    res = run_bass_kernel_spmd(
        nc, in_maps, core_ids=list(range(N_CORES)), trace=_trace
    )
    LAST_RESULT["exec_time_ns"] = res.exec_time_ns
    LAST_RESULT["res"] = res
    outs = [res.results[i]["y"] for i in range(N_CORES)]
    return np.concatenate(outs, axis=0).astype(np.float32)


# revision 21
# speedup vs baseline: 1.1176x; 1.0644x over previous
"""Multi-head attention (B=16, T=1024, D=768, H=12) on 8 TRN2 NeuronCores.

Strategy: pure data parallelism over the batch — each core computes full MHA
for 2 batch elements. No collectives.

Device kernel design (per core, bf16 compute / fp32 accumulate):
  - Host pre-transposes x to xT[b] = x[b].T ([D, T]) and pre-packs all weights
    in SBUF-ready layouts, cast to bf16.
  - Heads are processed in pairs (2 x HS = 128 = partition width).
  - qT/kT ([128, T], head pair stacked on partitions) come from
    matmul(lhsT=W_pair[dchunk, 128], rhs=xT[dchunk, T]) accumulated over D.
  - S^T[s, t] per head via row-tiled (tile_position) K=64 matmuls packing both
    heads of a pair into the 128-row PE array concurrently.
  - exp via ScalarE activation (scale=1/sqrt(HS) folded in, no max subtraction:
    |S|/8 <= ~3 for this data, exp is safe in fp32->bf16).
  - O^T = v_aug^T @ expS^T with v_aug = [v | ones]: row 64 of the PSUM result
    is the softmax denominator l[t] for free.
  - l -> 1/l via reciprocal_approx_fast (rows of several (h, th) batched on
    partitions via small DMA gathers), broadcast across partitions via a
    DRAM-bounce DMA, normalization as a single DVE multiply per (h, th).
  - y = O_all @ Wp + bp with lhsT = O_all^T (naturally produced above).
"""

import os
from contextlib import ExitStack

import numpy as np
import ml_dtypes

import concourse.bacc as bacc
import concourse.bass as bass
import concourse.mybir as mybir
import concourse.tile as tile
from concourse.bass_utils import run_bass_kernel_spmd

BF16 = ml_dtypes.bfloat16

# Full problem dims
B, T_FULL, D_FULL, H, HS = 16, 1024, 768, 12, 64
N_CORES = 8
NB = B // N_CORES  # batch elements per core


def build_mha_nc(nb, t, d, npair, trn_type="TRN2", variant="full"):
    """Build the Bass program for `nb` batch elements, seq len `t`, model dim
    `d`, `npair` head pairs (each pair = 128 partition lanes)."""
    P = 128
    KC = d // P              # contraction chunks over model dim
    SC = t // P              # s (key position) chunks
    NTH = max(1, t // 512)   # output-column groups for S/O matmuls
    TW = t // NTH            # width of each group (<= 512)
    TC = t // P              # t row chunks for v/y
    D2 = d // 2              # y-proj free-dim split (<= 512 fp32 psum)
    dpair = 2 * HS           # 128
    scale = 1.0 / np.sqrt(HS)

    f32 = mybir.dt.float32
    bf16 = mybir.dt.bfloat16
    AF = mybir.ActivationFunctionType

    nc = bacc.Bacc(trn_type, target_bir_lowering=False, debug=False)

    xt_d = nc.dram_tensor("xt", [nb, d, t], bf16, kind="ExternalInput").ap()
    wq_d = nc.dram_tensor("wq", [P, npair, KC, dpair], bf16, kind="ExternalInput").ap()
    wk_d = nc.dram_tensor("wk", [P, npair, KC, dpair], bf16, kind="ExternalInput").ap()
    wv_d = nc.dram_tensor("wv", [P, KC, npair * dpair], bf16, kind="ExternalInput").ap()
    wp_d = nc.dram_tensor("wp", [P, KC, d], bf16, kind="ExternalInput").ap()
    bqk_d = nc.dram_tensor("bqk", [P, npair, 2], f32, kind="ExternalInput").ap()
    bv_d = nc.dram_tensor("bv", [P, npair, dpair], bf16, kind="ExternalInput").ap()
    bp_d = nc.dram_tensor("bp", [P, d], f32, kind="ExternalInput").ap()
    y_d = nc.dram_tensor("y", [nb, t, d], f32, kind="ExternalOutput").ap()

    with TileOrExit(nc) as (tc, ctx):
        # ---- persistent weights (one bufs=1 pool; each tag allocated once) ----
        p_w = ctx.enter_context(tc.tile_pool(name="p_w", bufs=1))
        wq_sb = p_w.tile([P, npair, KC, dpair], bf16, tag="wq", name="wq_sb")
        wk_sb = p_w.tile([P, npair, KC, dpair], bf16, tag="wk", name="wk_sb")
        wv_sb = p_w.tile([P, KC, npair * dpair], bf16, tag="wv", name="wv_sb")
        wp_sb = p_w.tile([P, KC, d], bf16, tag="wp", name="wp_sb")
        bqk_sb = p_w.tile([P, npair, 2], f32, tag="bqk", name="bqk_sb")
        bv_sb = p_w.tile([P, npair, dpair], bf16, tag="bv", name="bv_sb")
        bp_sb = p_w.tile([P, d], f32, tag="bp", name="bp_sb")
        # weight loads ride the gpsimd DMA queue so the sync queue is free for
        # xt (first compute dependency); split by chunk for fine-grained deps
        for c in range(KC):
            nc.gpsimd.dma_start(wv_sb[:, c], wv_d[:, c])
        nc.gpsimd.dma_start(bv_sb[:], bv_d)
        for pr in range(npair):
            nc.gpsimd.dma_start(wq_sb[:, pr], wq_d[:, pr])
            nc.gpsimd.dma_start(wk_sb[:, pr], wk_d[:, pr])
        nc.gpsimd.dma_start(bqk_sb[:], bqk_d)
        nc.gpsimd.dma_start(wp_sb[:], wp_d)
        nc.gpsimd.dma_start(bp_sb[:], bp_d)

        # ---- pools ----
        p_xt = ctx.enter_context(tc.tile_pool(name="p_xt", bufs=2))
        p_vall = ctx.enter_context(tc.tile_pool(name="p_vall", bufs=2))
        p_qk = ctx.enter_context(tc.tile_pool(name="p_qk", bufs=2))
        p_es = ctx.enter_context(tc.tile_pool(name="p_es", bufs=2))
        p_oall = ctx.enter_context(tc.tile_pool(name="p_oall", bufs=1))
        p_norm = ctx.enter_context(tc.tile_pool(name="p_norm", bufs=2))
        p_y = ctx.enter_context(tc.tile_pool(name="p_y", bufs=2))
        p_dram = ctx.enter_context(tc.tile_pool(name="p_dram", bufs=2, space="DRAM"))
        ps_s = ctx.enter_context(tc.tile_pool(name="ps_s", bufs=2, space="PSUM"))
        ps_o = ctx.enter_context(tc.tile_pool(name="ps_o", bufs=2, space="PSUM"))
        ps_m = ctx.enter_context(tc.tile_pool(name="ps_m", bufs=2, space="PSUM"))

        for b in range(nb):
            xt = p_xt.tile([P, KC, t], bf16, tag="xt", name="xt_sb")
            xt_src = xt_d[b].rearrange("(c p) t -> p c t", p=P)
            for c in range(KC):
                nc.sync.dma_start(xt[:, c], xt_src[:, c])

            # ---- v projection: v_all[:, sc, pair, 0:65]   = [v_h0 | ones]
            #                    v_all[:, sc, pair, 65:130] = [v_h1 | ones]
            v_all = p_vall.tile([P, SC, npair, 130], bf16, tag="vall", name="v_all")
            ones_view = v_all.rearrange("p s r (h x) -> p s r h x", h=2)
            nc.gpsimd.memset(ones_view[:, :, :, :, 64:65], 1.0)
            nhalf = (npair + 2) // 3  # groups of <=3 pairs per psum tile
            for tci in range(TC):
                gns = [min(3, npair - 3 * g) for g in range(nhalf)]
                psvs = [ps_m.tile([P, 3 * dpair], f32, tag="m", name="psv") for _ in range(nhalf)]
                for c in range(KC):
                    for g in range(nhalf):
                        nc.tensor.matmul(
                            psvs[g][:, : gns[g] * dpair],
                            lhsT=xt[:, c, tci * P : (tci + 1) * P],
                            rhs=wv_sb[:, c, 3 * g * dpair : (3 * g + gns[g]) * dpair],
                            start=(c == 0),
                            stop=(c == KC - 1),
                        )
                for g in range(nhalf):
                    glo, gn = 3 * g, gns[g]
                    dst = v_all[:, tci, glo : glo + gn, :].rearrange(
                        "p r (h x) -> p r h x", h=2
                    )[:, :, :, 0:64]
                    src = psvs[g][:, : gn * dpair].rearrange("p (r h e) -> p r h e", r=gn, h=2)
                    bias = bv_sb[:, glo : glo + gn, :].rearrange("p r (h e) -> p r h e", h=2)
                    nc.vector.tensor_add(out=dst, in0=src, in1=bias)

            o_allT = p_oall.tile([P, npair, t], bf16, tag="oall", name="o_allT")

            for pr in range(npair):
                # ---- q/k head-pair projections -> qT/kT [128, t] bf16
                qT = p_qk.tile([P, t], bf16, tag="qT", name="qT")
                kT = p_qk.tile([P, t], bf16, tag="kT", name="kT")
                for w_sb, bj, dstT in ((wq_sb, 0, qT), (wk_sb, 1, kT)):
                    psqs = [ps_m.tile([P, TW], f32, tag="m", name="psq") for _ in range(NTH)]
                    for c in range(KC):
                        for th in range(NTH):
                            nc.tensor.matmul(
                                psqs[th][:],
                                lhsT=w_sb[:, pr, c, :],
                                rhs=xt[:, c, th * TW : (th + 1) * TW],
                                start=(c == 0),
                                stop=(c == KC - 1),
                            )
                    for th in range(NTH):
                        nc.vector.tensor_scalar_add(
                            out=dstT[:, th * TW : (th + 1) * TW],
                            in0=psqs[th][:],
                            scalar1=bqk_sb[:, pr, bj : bj + 1],
                        )

                # ---- fused S -> exp -> O pipeline per t-half, with one-stage
                # skew (S(sc+1) emitted before O(sc)) and qk(p+1) dripped in
                for th in range(NTH):
                    es = p_es.tile([P, SC, 2, TW], bf16, tag="es", name="es")
                    psos = [ps_o.tile([65, TW], f32, tag="o", name="pso") for _ in range(2)]
                    for sc in range(SC + 1):
                        if sc < SC:
                            ps = ps_s.tile([P, 2, TW], f32, tag="s", name="ps_s")
                            nc.tensor.matmul(
                                ps[:, 0, :],
                                lhsT=kT[0:64, sc * P : (sc + 1) * P],
                                rhs=qT[0:64, th * TW : (th + 1) * TW],
                                start=True,
                                stop=True,
                            )
                            nc.tensor.matmul(
                                ps[:, 1, :],
                                lhsT=kT[64:128, sc * P : (sc + 1) * P],
                                rhs=qT[64:128, th * TW : (th + 1) * TW],
                                start=True,
                                stop=True,
                                tile_position=None if "notile" in variant else (64, 0),
                            )
                            nc.scalar.activation(
                                out=es[:, sc, :, :], in_=ps[:], func=AF.Exp, scale=scale
                            )
                        if sc >= 1:
                            so = sc - 1
                            for h in range(2):
                                nc.tensor.matmul(
                                    psos[h][:],
                                    lhsT=v_all[:, so, pr, 65 * h : 65 * h + 65],
                                    rhs=es[:, so, h, :],
                                    start=(so == 0),
                                    stop=(so == SC - 1),
                                )
                    # gather the two l rows, invert, broadcast, normalize
                    l_sb = p_norm.tile([65, 2, TW], f32, tag="l", name="l_sb")
                    for h in range(2):
                        nc.vector.tensor_copy(out=l_sb[64:65, h, :], in_=psos[h][64:65, :])
                    # both l rows parked on partition 0 (different free offsets):
                    # partition_broadcast reads physical partition 0 on HW
                    lg = p_norm.tile([1, 2, TW], f32, tag="lg", name="lg")
                    for h in range(2):
                        nc.sync.dma_start(out=lg[0:1, h, :], in_=l_sb[64:65, h, :])
                    lginv = p_norm.tile([1, 2, TW], f32, tag="lginv", name="lginv")
                    if "norecip" in variant:
                        nc.vector.tensor_copy(out=lginv[:], in_=lg[:])
                    else:
                        nc.vector.reciprocal_approx_fast(out=lginv[:], in_=lg[:])
                    linv = p_norm.tile([64, 2, TW], f32, tag="linv", name="linv")
                    if "nobcast" in variant:
                        nc.vector.memset(linv[:], 1.0)
                    else:
                        for h in range(2):
                            nc.gpsimd.partition_broadcast(
                                out_ap=linv[:, h, :],
                                in_ap=lginv[0:1, h, :],
                                channels=64,
                            )
                    for h in range(2):
                        if h == 0:
                            nc.vector.tensor_mul(
                                out=o_allT[0:64, pr, th * TW : (th + 1) * TW],
                                in0=psos[h][0:64, :],
                                in1=linv[:, h, :],
                            )
                        else:
                            ot = p_norm.tile([64, TW], bf16, tag="ot", name="ot")
                            nc.vector.tensor_mul(out=ot[:], in0=psos[h][0:64, :], in1=linv[:, h, :])
                            nc.sync.dma_start(
                                out=o_allT[64:128, pr, th * TW : (th + 1) * TW], in_=ot[:]
                            )
            # ---- output projection y = O_all @ Wp + bp
            for tci in range(TC):
                psy = [ps_o.tile([P, D2], f32, tag="o", name="psy") for _ in range(2)]
                for c in range(KC):
                    for j in range(2):
                        nc.tensor.matmul(
                            psy[j][:],
                            lhsT=o_allT[:, c, tci * P : (tci + 1) * P],
                            rhs=wp_sb[:, c, j * D2 : (j + 1) * D2],
                            start=(c == 0),
                            stop=(c == KC - 1),
                        )
                y_sb = p_y.tile([P, d], f32, tag="y", name="y_sb")
                for j in range(2):
                    nc.vector.tensor_add(
                        out=y_sb[:, j * D2 : (j + 1) * D2],
                        in0=psy[j][:],
                        in1=bp_sb[:, j * D2 : (j + 1) * D2],
                    )
                nc.sync.dma_start(out=y_d[b, tci * P : (tci + 1) * P, :], in_=y_sb[:])

    nc.compile()
    return nc


class TileOrExit:
    """Combined TileContext + ExitStack context manager."""

    def __init__(self, nc):
        self.nc = nc
        self.ctx = ExitStack()
        self.tc = tile.TileContext(nc)

    def __enter__(self):
        self.ctx.__enter__()
        self.tc.__enter__()
        return self.tc, self.ctx

    def __exit__(self, *a):
        # close pools before TileContext exits scheduling
        self.ctx.__exit__(*a)
        return self.tc.__exit__(*a)


def prep_inputs(x, Wq, bq, Wk, bk, Wv, bv, Wp, bp, nb, npair):
    """Host-side packing into the DRAM layouts the device kernel expects.

    Returns (shared weight map, list of per-core input maps)."""
    P = 128
    t = x.shape[1]
    d = x.shape[2]
    KC = d // P
    dpair = 2 * HS

    def to_bf(a):
        return np.ascontiguousarray(a).astype(BF16)

    # x^T per batch element
    xt = np.ascontiguousarray(x.transpose(0, 2, 1)).astype(BF16)  # [B, d, t]

    # wq/wk: [P, pair, c, 128] with cols 0:64 = head 2p, 64:128 = head 2p+1
    def pack_qk(W):
        # W: [H, d, HS] -> [pair, 2, KC, P, HS] -> [P, pair, KC, 2*HS]
        w = W.reshape(npair, 2, KC, P, HS)
        w = w.transpose(3, 0, 2, 1, 4).reshape(P, npair, KC, dpair)
        return to_bf(w)

    wq = pack_qk(Wq)
    wk = pack_qk(Wk)
    wv = pack_qk(Wv).transpose(0, 2, 1, 3).reshape(P, KC, npair * dpair)
    wv = np.ascontiguousarray(wv)  # [P, c, pair*128]
    # wp: [P, c, d]
    wp = to_bf(Wp.reshape(KC, P, d).transpose(1, 0, 2))
    # bqk: [P, pair, 2] fp32: partition = pair-stacked head dims
    bqk = np.stack(
        [bq.reshape(npair, dpair), bk.reshape(npair, dpair)], axis=-1
    )  # [pair, 128, 2]
    bqk = np.ascontiguousarray(bqk.transpose(1, 0, 2)).astype(np.float32)  # [P, pair, 2]
    # bv broadcast along t partitions: [P, pair, 128]
    bv_bc = np.broadcast_to(bv.reshape(1, npair, dpair), (P, npair, dpair))
    bv_bc = to_bf(bv_bc)
    # bp broadcast: [P, d] fp32
    bp_bc = np.ascontiguousarray(np.broadcast_to(bp.reshape(1, d), (P, d))).astype(
        np.float32
    )

    weights = {
        "wq": wq,
        "wk": wk,
        "wv": wv,
        "wp": wp,
        "bqk": bqk,
        "bv": bv_bc,
        "bp": bp_bc,
    }
    n_cores = x.shape[0] // nb
    in_maps = []
    for i in range(n_cores):
        m = dict(weights)
        m["xt"] = np.ascontiguousarray(xt[i * nb : (i + 1) * nb])
        in_maps.append(m)
    return in_maps


_NC_CACHE = {}
LAST_RESULT = {}


def kernel(x, Wq, bq, Wk, bk, Wv, bv, Wp, bp, _trace=False):
    x = np.asarray(x, dtype=np.float32)
    Wq, bq = np.asarray(Wq, np.float32), np.asarray(bq, np.float32)
    Wk, bk = np.asarray(Wk, np.float32), np.asarray(bk, np.float32)
    Wv, bv = np.asarray(Wv, np.float32), np.asarray(bv, np.float32)
    Wp, bp = np.asarray(Wp, np.float32), np.asarray(bp, np.float32)

    npair = H // 2
    key = ("full", NB, T_FULL, D_FULL, npair)
    if key not in _NC_CACHE:
        _NC_CACHE[key] = build_mha_nc(NB, T_FULL, D_FULL, npair)
    nc = _NC_CACHE[key]

    in_maps = prep_inputs(x, Wq, bq, Wk, bk, Wv, bv, Wp, bp, NB, npair)
    res = run_bass_kernel_spmd(
        nc, in_maps, core_ids=list(range(N_CORES)), trace=_trace
    )
    LAST_RESULT["exec_time_ns"] = res.exec_time_ns
    LAST_RESULT["res"] = res
    outs = [res.results[i]["y"] for i in range(N_CORES)]
    return np.concatenate(outs, axis=0).astype(np.float32)


# revision 29
# speedup vs baseline: 1.2918x; 1.1558x over previous
"""Multi-head attention (B=16, T=1024, D=768, H=12) on 8 TRN2 NeuronCores.

Strategy: pure data parallelism over the batch — each core computes full MHA
for 2 batch elements. No collectives.

Device kernel design (per core, bf16 compute / fp32 accumulate):
  - Host pre-transposes x to xT[b] = x[b].T ([D, T]) and pre-packs all weights
    in SBUF-ready layouts, cast to bf16.
  - Heads are processed in pairs (2 x HS = 128 = partition width).
  - qT/kT ([128, T], head pair stacked on partitions) come from
    matmul(lhsT=W_pair[dchunk, 128], rhs=xT[dchunk, T]) accumulated over D.
  - S^T[s, t] per head via row-tiled (tile_position) K=64 matmuls packing both
    heads of a pair into the 128-row PE array concurrently.
  - exp via ScalarE activation (scale=1/sqrt(HS) folded in, no max subtraction:
    |S|/8 <= ~3 for this data, exp is safe in fp32->bf16).
  - O^T = v_aug^T @ expS^T with v_aug = [v | ones]: row 64 of the PSUM result
    is the softmax denominator l[t] for free.
  - l -> 1/l via reciprocal_approx_fast (rows of several (h, th) batched on
    partitions via small DMA gathers), broadcast across partitions via a
    DRAM-bounce DMA, normalization as a single DVE multiply per (h, th).
  - y = O_all @ Wp + bp with lhsT = O_all^T (naturally produced above).
"""

import os
from contextlib import ExitStack

import numpy as np
import ml_dtypes

import concourse.bacc as bacc
import concourse.bass as bass
import concourse.mybir as mybir
import concourse.tile as tile
from concourse.bass_utils import run_bass_kernel_spmd

BF16 = ml_dtypes.bfloat16

# Full problem dims
B, T_FULL, D_FULL, H, HS = 16, 1024, 768, 12, 64
N_CORES = 8
NB = B // N_CORES  # batch elements per core


def build_mha_nc(nb, t, d, npair, trn_type="TRN2", variant="full"):
    """Build the Bass program for `nb` batch elements, seq len `t`, model dim
    `d`, `npair` head pairs (each pair = 128 partition lanes)."""
    P = 128
    KC = d // P              # contraction chunks over model dim
    SC = t // P              # s (key position) chunks
    NTH = max(1, t // 512)   # output-column groups for S/O matmuls
    TW = t // NTH            # width of each group (<= 512)
    TC = t // P              # t row chunks for v/y
    D2 = d // 2              # y-proj free-dim split (<= 512 fp32 psum)
    dpair = 2 * HS           # 128
    scale = 1.0 / np.sqrt(HS)

    f32 = mybir.dt.float32
    bf16 = mybir.dt.bfloat16
    AF = mybir.ActivationFunctionType

    nc = bacc.Bacc(trn_type, target_bir_lowering=False, debug=False)

    xt_d = nc.dram_tensor("xt", [nb, d, t], bf16, kind="ExternalInput").ap()
    wq_d = nc.dram_tensor("wq", [P, npair, KC, dpair], bf16, kind="ExternalInput").ap()
    wk_d = nc.dram_tensor("wk", [P, npair, KC, dpair], bf16, kind="ExternalInput").ap()
    wv_d = nc.dram_tensor("wv", [P, KC, npair * dpair], bf16, kind="ExternalInput").ap()
    wp_d = nc.dram_tensor("wp", [P, KC, d], bf16, kind="ExternalInput").ap()
    bqk_d = nc.dram_tensor("bqk", [P, npair, 2], f32, kind="ExternalInput").ap()
    bv_d = nc.dram_tensor("bv", [P, npair, dpair], bf16, kind="ExternalInput").ap()
    bp_d = nc.dram_tensor("bp", [P, d], f32, kind="ExternalInput").ap()
    y_d = nc.dram_tensor("y", [nb, t, d], f32, kind="ExternalOutput").ap()

    with TileOrExit(nc) as (tc, ctx):
        # ---- persistent weights (one bufs=1 pool; each tag allocated once) ----
        p_w = ctx.enter_context(tc.tile_pool(name="p_w", bufs=1))
        wq_sb = p_w.tile([P, npair, KC, dpair], bf16, tag="wq", name="wq_sb")
        wk_sb = p_w.tile([P, npair, KC, dpair], bf16, tag="wk", name="wk_sb")
        wv_sb = p_w.tile([P, KC, npair * dpair], bf16, tag="wv", name="wv_sb")
        wp_sb = p_w.tile([P, KC, d], bf16, tag="wp", name="wp_sb")
        bqk_sb = p_w.tile([P, npair, 2], f32, tag="bqk", name="bqk_sb")
        bv_sb = p_w.tile([P, npair, dpair], bf16, tag="bv", name="bv_sb")
        bp_sb = p_w.tile([P, d], f32, tag="bp", name="bp_sb")
        # weight loads ride the gpsimd DMA queue so the sync queue is free for
        # xt (first compute dependency); split by chunk for fine-grained deps
        for c in range(KC):
            nc.gpsimd.dma_start(wv_sb[:, c], wv_d[:, c])
        nc.gpsimd.dma_start(bv_sb[:], bv_d)
        for pr in range(npair):
            nc.gpsimd.dma_start(wq_sb[:, pr], wq_d[:, pr])
            nc.gpsimd.dma_start(wk_sb[:, pr], wk_d[:, pr])
        nc.gpsimd.dma_start(bqk_sb[:], bqk_d)
        nc.gpsimd.dma_start(wp_sb[:], wp_d)
        nc.gpsimd.dma_start(bp_sb[:], bp_d)

        # ---- pools ----
        p_xt = ctx.enter_context(tc.tile_pool(name="p_xt", bufs=2))
        p_vall = ctx.enter_context(tc.tile_pool(name="p_vall", bufs=2))
        p_qk = ctx.enter_context(tc.tile_pool(name="p_qk", bufs=2))
        p_es = ctx.enter_context(tc.tile_pool(name="p_es", bufs=3))
        p_oall = ctx.enter_context(tc.tile_pool(name="p_oall", bufs=1))
        p_norm = ctx.enter_context(tc.tile_pool(name="p_norm", bufs=2))
        p_y = ctx.enter_context(tc.tile_pool(name="p_y", bufs=2))
        p_dram = ctx.enter_context(tc.tile_pool(name="p_dram", bufs=2, space="DRAM"))
        ps_s = ctx.enter_context(tc.tile_pool(name="ps_s", bufs=2, space="PSUM"))
        ps_o = ctx.enter_context(tc.tile_pool(name="ps_o", bufs=2, space="PSUM"))
        ps_m = ctx.enter_context(tc.tile_pool(name="ps_m", bufs=2, space="PSUM"))

        # HAM warm-up: a burst of dummy matmuls during the initial DMA wait
        # so the PE clock is at 2.4 GHz when real work arrives
        warm = p_norm.tile([P, TW], bf16, tag="warm", name="warm")
        nc.vector.memset(warm[:], 0.0)
        wps = ps_m.tile([P, TW], f32, tag="m", name="wps")
        for i in range(12):
            nc.tensor.matmul(
                wps[:], lhsT=warm[:, 0:P], rhs=warm[:], start=(i == 0), stop=(i == 11)
            )

        for b in range(nb):
            xt = p_xt.tile([P, KC, t], bf16, tag="xt", name="xt_sb")
            xt_src = xt_d[b].rearrange("(c p) t -> p c t", p=P)
            for c in range(KC):
                nc.sync.dma_start(xt[:, c], xt_src[:, c])

            # ---- v projection: v_all[:, sc, pair, 0:65]   = [v_h0 | ones]
            #                    v_all[:, sc, pair, 65:130] = [v_h1 | ones]
            v_all = p_vall.tile([P, SC, npair, 130], bf16, tag="vall", name="v_all")
            ones_view = v_all.rearrange("p s r (h x) -> p s r h x", h=2)
            nc.gpsimd.memset(ones_view[:, :, :, :, 64:65], 1.0)
            nhalf = (npair + 2) // 3  # groups of <=3 pairs per psum tile
            for tci in range(TC):
                gns = [min(3, npair - 3 * g) for g in range(nhalf)]
                psvs = [ps_m.tile([P, 3 * dpair], f32, tag="m", name="psv") for _ in range(nhalf)]
                for c in range(KC):
                    for g in range(nhalf):
                        nc.tensor.matmul(
                            psvs[g][:, : gns[g] * dpair],
                            lhsT=xt[:, c, tci * P : (tci + 1) * P],
                            rhs=wv_sb[:, c, 3 * g * dpair : (3 * g + gns[g]) * dpair],
                            start=(c == 0),
                            stop=(c == KC - 1),
                        )
                for g in range(nhalf):
                    glo, gn = 3 * g, gns[g]
                    dst = v_all[:, tci, glo : glo + gn, :].rearrange(
                        "p r (h x) -> p r h x", h=2
                    )[:, :, :, 0:64]
                    src = psvs[g][:, : gn * dpair].rearrange("p (r h e) -> p r h e", r=gn, h=2)
                    bias = bv_sb[:, glo : glo + gn, :].rearrange("p r (h e) -> p r h e", h=2)
                    nc.vector.tensor_add(out=dst, in0=src, in1=bias)

            o_allT = p_oall.tile([P, npair, t], bf16, tag="oall", name="o_allT")

            for pr in range(npair):
                # ---- q/k head-pair projections -> qT/kT [128, t] bf16
                qT = p_qk.tile([P, t], bf16, tag="qT", name="qT")
                kT = p_qk.tile([P, t], bf16, tag="kT", name="kT")
                for w_sb, bj, dstT in ((wq_sb, 0, qT), (wk_sb, 1, kT)):
                    psqs = [ps_m.tile([P, TW], f32, tag="m", name="psq") for _ in range(NTH)]
                    for c in range(KC):
                        for th in range(NTH):
                            nc.tensor.matmul(
                                psqs[th][:],
                                lhsT=w_sb[:, pr, c, :],
                                rhs=xt[:, c, th * TW : (th + 1) * TW],
                                start=(c == 0),
                                stop=(c == KC - 1),
                            )
                    for th in range(NTH):
                        nc.vector.tensor_scalar_add(
                            out=dstT[:, th * TW : (th + 1) * TW],
                            in0=psqs[th][:],
                            scalar1=bqk_sb[:, pr, bj : bj + 1],
                        )

                # ---- fused S -> exp -> O pipeline per t-half, with one-stage
                # skew (S(sc+1) emitted before O(sc)) and qk(p+1) dripped in
                for th in range(NTH):
                    es = p_es.tile([P, SC, 2, TW], bf16, tag="es", name="es")
                    psos = [ps_o.tile([65, TW], f32, tag="o", name="pso") for _ in range(2)]
                    for sc in range(SC + 2):
                        if sc < SC:
                            ps = ps_s.tile([P, 2, TW], f32, tag="s", name="ps_s")
                            nc.tensor.matmul(
                                ps[:, 0, :],
                                lhsT=kT[0:64, sc * P : (sc + 1) * P],
                                rhs=qT[0:64, th * TW : (th + 1) * TW],
                                start=True,
                                stop=True,
                            )
                            nc.tensor.matmul(
                                ps[:, 1, :],
                                lhsT=kT[64:128, sc * P : (sc + 1) * P],
                                rhs=qT[64:128, th * TW : (th + 1) * TW],
                                start=True,
                                stop=True,
                                tile_position=None if "notile" in variant else (64, 0),
                            )
                            nc.scalar.activation(
                                out=es[:, sc, :, :], in_=ps[:], func=AF.Exp, scale=scale
                            )
                        if sc >= 2:
                            so = sc - 2
                            for h in range(2):
                                nc.tensor.matmul(
                                    psos[h][:],
                                    lhsT=v_all[:, so, pr, 65 * h : 65 * h + 65],
                                    rhs=es[:, so, h, :],
                                    start=(so == 0),
                                    stop=(so == SC - 1),
                                )
                    # invert the l rows straight out of PSUM (same lane 64),
                    # then DMA the reciprocals down to partition 0 where
                    # partition_broadcast can read them
                    l_sb = p_norm.tile([65, 2, TW], f32, tag="l", name="l_sb")
                    for h in range(2):
                        nc.vector.tensor_copy(out=l_sb[64:65, h, :], in_=psos[h][64:65, :])
                    lg = p_norm.tile([1, 2, TW], f32, tag="lg", name="lg")
                    nc.sync.dma_start(out=lg[0:1, :, :], in_=l_sb[64:65, :, :])
                    lginv = p_norm.tile([1, 2, TW], f32, tag="lginv", name="lginv")
                    if "norecip" in variant:
                        nc.vector.tensor_copy(out=lginv[:], in_=lg[:])
                    else:
                        # custom DVE ops only work at base partition 0 on HW
                        nc.vector.reciprocal_approx_fast(out=lginv[:], in_=lg[:])
                    linv = p_norm.tile([64, 2, TW], f32, tag="linv", name="linv")
                    if "nobcast" in variant:
                        nc.vector.memset(linv[:], 1.0)
                    else:
                        for h in range(2):
                            nc.gpsimd.partition_broadcast(
                                out_ap=linv[:, h, :],
                                in_ap=lginv[0:1, h, :],
                                channels=64,
                            )
                    for h in range(2):
                        if h == 0:
                            nc.vector.tensor_mul(
                                out=o_allT[0:64, pr, th * TW : (th + 1) * TW],
                                in0=psos[h][0:64, :],
                                in1=linv[:, h, :],
                            )
                        else:
                            ot = p_norm.tile([64, TW], bf16, tag="ot", name="ot")
                            nc.vector.tensor_mul(out=ot[:], in0=psos[h][0:64, :], in1=linv[:, h, :])
                            nc.sync.dma_start(
                                out=o_allT[64:128, pr, th * TW : (th + 1) * TW], in_=ot[:]
                            )
            # ---- output projection y = O_all @ Wp + bp
            # last b: ps_m is idle during the final pair's chains (no next-pair
            # qk), so y can start accumulating early pairs there; earlier b's
            # keep y on ps_o so it doesn't contend with the next b's v-proj
            pool_y, tag_y = (ps_m, "m") if b == nb - 1 else (ps_o, "o")
            for tci in range(TC):
                psy = [pool_y.tile([P, D2], f32, tag=tag_y, name="psy") for _ in range(2)]
                for c in range(KC):
                    for j in range(2):
                        nc.tensor.matmul(
                            psy[j][:],
                            lhsT=o_allT[:, c, tci * P : (tci + 1) * P],
                            rhs=wp_sb[:, c, j * D2 : (j + 1) * D2],
                            start=(c == 0),
                            stop=(c == KC - 1),
                        )
                y_sb = p_y.tile([P, d], f32, tag="y", name="y_sb")
                for j in range(2):
                    nc.vector.tensor_add(
                        out=y_sb[:, j * D2 : (j + 1) * D2],
                        in0=psy[j][:],
                        in1=bp_sb[:, j * D2 : (j + 1) * D2],
                    )
                nc.sync.dma_start(out=y_d[b, tci * P : (tci + 1) * P, :], in_=y_sb[:])

    nc.compile()
    return nc


class TileOrExit:
    """Combined TileContext + ExitStack context manager."""

    def __init__(self, nc):
        self.nc = nc
        self.ctx = ExitStack()
        self.tc = tile.TileContext(nc)

    def __enter__(self):
        self.ctx.__enter__()
        self.tc.__enter__()
        return self.tc, self.ctx

    def __exit__(self, *a):
        # close pools before TileContext exits scheduling
        self.ctx.__exit__(*a)
        return self.tc.__exit__(*a)


def prep_inputs(x, Wq, bq, Wk, bk, Wv, bv, Wp, bp, nb, npair):
    """Host-side packing into the DRAM layouts the device kernel expects.

    Returns (shared weight map, list of per-core input maps)."""
    P = 128
    t = x.shape[1]
    d = x.shape[2]
    KC = d // P
    dpair = 2 * HS

    def to_bf(a):
        return np.ascontiguousarray(a).astype(BF16)

    # x^T per batch element
    xt = np.ascontiguousarray(x.transpose(0, 2, 1)).astype(BF16)  # [B, d, t]

    # wq/wk: [P, pair, c, 128] with cols 0:64 = head 2p, 64:128 = head 2p+1
    def pack_qk(W):
        # W: [H, d, HS] -> [pair, 2, KC, P, HS] -> [P, pair, KC, 2*HS]
        w = W.reshape(npair, 2, KC, P, HS)
        w = w.transpose(3, 0, 2, 1, 4).reshape(P, npair, KC, dpair)
        return to_bf(w)

    wq = pack_qk(Wq)
    wk = pack_qk(Wk)
    wv = pack_qk(Wv).transpose(0, 2, 1, 3).reshape(P, KC, npair * dpair)
    wv = np.ascontiguousarray(wv)  # [P, c, pair*128]
    # wp: [P, c, d]
    wp = to_bf(Wp.reshape(KC, P, d).transpose(1, 0, 2))
    # bqk: [P, pair, 2] fp32: partition = pair-stacked head dims
    bqk = np.stack(
        [bq.reshape(npair, dpair), bk.reshape(npair, dpair)], axis=-1
    )  # [pair, 128, 2]
    bqk = np.ascontiguousarray(bqk.transpose(1, 0, 2)).astype(np.float32)  # [P, pair, 2]
    # bv broadcast along t partitions: [P, pair, 128]
    bv_bc = np.broadcast_to(bv.reshape(1, npair, dpair), (P, npair, dpair))
    bv_bc = to_bf(bv_bc)
    # bp broadcast: [P, d] fp32
    bp_bc = np.ascontiguousarray(np.broadcast_to(bp.reshape(1, d), (P, d))).astype(
        np.float32
    )

    weights = {
        "wq": wq,
        "wk": wk,
        "wv": wv,
        "wp": wp,
        "bqk": bqk,
        "bv": bv_bc,
        "bp": bp_bc,
    }
    n_cores = x.shape[0] // nb
    in_maps = []
    for i in range(n_cores):
        m = dict(weights)
        m["xt"] = np.ascontiguousarray(xt[i * nb : (i + 1) * nb])
        in_maps.append(m)
    return in_maps


_NC_CACHE = {}
LAST_RESULT = {}


def kernel(x, Wq, bq, Wk, bk, Wv, bv, Wp, bp, _trace=False):
    x = np.asarray(x, dtype=np.float32)
    Wq, bq = np.asarray(Wq, np.float32), np.asarray(bq, np.float32)
    Wk, bk = np.asarray(Wk, np.float32), np.asarray(bk, np.float32)
    Wv, bv = np.asarray(Wv, np.float32), np.asarray(bv, np.float32)
    Wp, bp = np.asarray(Wp, np.float32), np.asarray(bp, np.float32)

    npair = H // 2
    key = ("full", NB, T_FULL, D_FULL, npair)
    if key not in _NC_CACHE:
        _NC_CACHE[key] = build_mha_nc(NB, T_FULL, D_FULL, npair)
    nc = _NC_CACHE[key]

    in_maps = prep_inputs(x, Wq, bq, Wk, bk, Wv, bv, Wp, bp, NB, npair)
    res = run_bass_kernel_spmd(
        nc, in_maps, core_ids=list(range(N_CORES)), trace=_trace
    )
    LAST_RESULT["exec_time_ns"] = res.exec_time_ns
    LAST_RESULT["res"] = res
    outs = [res.results[i]["y"] for i in range(N_CORES)]
    return np.concatenate(outs, axis=0).astype(np.float32)


# revision 30
# speedup vs baseline: 1.2986x; 1.0053x over previous
"""Multi-head attention (B=16, T=1024, D=768, H=12) on 8 TRN2 NeuronCores.

Strategy: pure data parallelism over the batch — each core computes full MHA
for 2 batch elements. No collectives.

Device kernel design (per core, bf16 compute / fp32 accumulate):
  - Host pre-transposes x to xT[b] = x[b].T ([D, T]) and pre-packs all weights
    in SBUF-ready layouts, cast to bf16.
  - Heads are processed in pairs (2 x HS = 128 = partition width).
  - qT/kT ([128, T], head pair stacked on partitions) come from
    matmul(lhsT=W_pair[dchunk, 128], rhs=xT[dchunk, T]) accumulated over D.
  - S^T[s, t] per head via row-tiled (tile_position) K=64 matmuls packing both
    heads of a pair into the 128-row PE array concurrently.
  - exp via ScalarE activation (scale=1/sqrt(HS) folded in, no max subtraction:
    |S|/8 <= ~3 for this data, exp is safe in fp32->bf16).
  - O^T = v_aug^T @ expS^T with v_aug = [v | ones]: row 64 of the PSUM result
    is the softmax denominator l[t] for free.
  - l -> 1/l via reciprocal_approx_fast (rows of several (h, th) batched on
    partitions via small DMA gathers), broadcast across partitions via a
    DRAM-bounce DMA, normalization as a single DVE multiply per (h, th).
  - y = O_all @ Wp + bp with lhsT = O_all^T (naturally produced above).
"""

import os
from contextlib import ExitStack

import numpy as np
import ml_dtypes

import concourse.bacc as bacc
import concourse.bass as bass
import concourse.mybir as mybir
import concourse.tile as tile
from concourse.bass_utils import run_bass_kernel_spmd

BF16 = ml_dtypes.bfloat16

# Full problem dims
B, T_FULL, D_FULL, H, HS = 16, 1024, 768, 12, 64
N_CORES = 8
NB = B // N_CORES  # batch elements per core


def build_mha_nc(nb, t, d, npair, trn_type="TRN2", variant="full"):
    """Build the Bass program for `nb` batch elements, seq len `t`, model dim
    `d`, `npair` head pairs (each pair = 128 partition lanes)."""
    P = 128
    KC = d // P              # contraction chunks over model dim
    SC = t // P              # s (key position) chunks
    NTH = max(1, t // 512)   # output-column groups for S/O matmuls
    TW = t // NTH            # width of each group (<= 512)
    TC = t // P              # t row chunks for v/y
    D2 = d // 2              # y-proj free-dim split (<= 512 fp32 psum)
    dpair = 2 * HS           # 128
    scale = 1.0 / np.sqrt(HS)

    f32 = mybir.dt.float32
    bf16 = mybir.dt.bfloat16
    AF = mybir.ActivationFunctionType

    nc = bacc.Bacc(trn_type, target_bir_lowering=False, debug=False)

    xt_d = nc.dram_tensor("xt", [nb, d, t], bf16, kind="ExternalInput").ap()
    wq_d = nc.dram_tensor("wq", [P, npair, KC, dpair], bf16, kind="ExternalInput").ap()
    wk_d = nc.dram_tensor("wk", [P, npair, KC, dpair], bf16, kind="ExternalInput").ap()
    wv_d = nc.dram_tensor("wv", [P, KC, npair * dpair], bf16, kind="ExternalInput").ap()
    wp_d = nc.dram_tensor("wp", [P, KC, d], bf16, kind="ExternalInput").ap()
    bqk_d = nc.dram_tensor("bqk", [P, npair, 2], f32, kind="ExternalInput").ap()
    bv_d = nc.dram_tensor("bv", [P, npair, dpair], bf16, kind="ExternalInput").ap()
    bp_d = nc.dram_tensor("bp", [P, d], f32, kind="ExternalInput").ap()
    y_d = nc.dram_tensor("y", [nb, t, d], f32, kind="ExternalOutput").ap()

    with TileOrExit(nc) as (tc, ctx):
        # ---- persistent weights (one bufs=1 pool; each tag allocated once) ----
        p_w = ctx.enter_context(tc.tile_pool(name="p_w", bufs=1))
        wq_sb = p_w.tile([P, npair, KC, dpair], bf16, tag="wq", name="wq_sb")
        wk_sb = p_w.tile([P, npair, KC, dpair], bf16, tag="wk", name="wk_sb")
        wv_sb = p_w.tile([P, KC, npair * dpair], bf16, tag="wv", name="wv_sb")
        wp_sb = p_w.tile([P, KC, d], bf16, tag="wp", name="wp_sb")
        bqk_sb = p_w.tile([P, npair, 2], f32, tag="bqk", name="bqk_sb")
        bv_sb = p_w.tile([P, npair, dpair], bf16, tag="bv", name="bv_sb")
        bp_sb = p_w.tile([P, d], f32, tag="bp", name="bp_sb")
        # weight loads ride the gpsimd DMA queue so the sync queue is free for
        # xt (first compute dependency); split by chunk for fine-grained deps
        for c in range(KC):
            nc.gpsimd.dma_start(wv_sb[:, c], wv_d[:, c])
        nc.gpsimd.dma_start(bv_sb[:], bv_d)
        for pr in range(npair):
            nc.gpsimd.dma_start(wq_sb[:, pr], wq_d[:, pr])
            nc.gpsimd.dma_start(wk_sb[:, pr], wk_d[:, pr])
        nc.gpsimd.dma_start(bqk_sb[:], bqk_d)
        nc.gpsimd.dma_start(wp_sb[:], wp_d)
        nc.gpsimd.dma_start(bp_sb[:], bp_d)

        # ---- pools ----
        p_xt = ctx.enter_context(tc.tile_pool(name="p_xt", bufs=2))
        p_vall = ctx.enter_context(tc.tile_pool(name="p_vall", bufs=2))
        p_qk = ctx.enter_context(tc.tile_pool(name="p_qk", bufs=3))
        p_es = ctx.enter_context(tc.tile_pool(name="p_es", bufs=3))
        p_oall = ctx.enter_context(tc.tile_pool(name="p_oall", bufs=1))
        p_norm = ctx.enter_context(tc.tile_pool(name="p_norm", bufs=2))
        p_y = ctx.enter_context(tc.tile_pool(name="p_y", bufs=2))
        p_dram = ctx.enter_context(tc.tile_pool(name="p_dram", bufs=2, space="DRAM"))
        ps_s = ctx.enter_context(tc.tile_pool(name="ps_s", bufs=2, space="PSUM"))
        ps_o = ctx.enter_context(tc.tile_pool(name="ps_o", bufs=2, space="PSUM"))
        ps_m = ctx.enter_context(tc.tile_pool(name="ps_m", bufs=2, space="PSUM"))

        # HAM warm-up: a burst of dummy matmuls during the initial DMA wait
        # so the PE clock is at 2.4 GHz when real work arrives
        warm = p_norm.tile([P, TW], bf16, tag="warm", name="warm")
        nc.vector.memset(warm[:], 0.0)
        wps = ps_m.tile([P, TW], f32, tag="m", name="wps")
        for i in range(24):
            nc.tensor.matmul(
                wps[:], lhsT=warm[:, 0:P], rhs=warm[:], start=(i == 0), stop=(i == 23)
            )

        for b in range(nb):
            xt = p_xt.tile([P, KC, t], bf16, tag="xt", name="xt_sb")
            xt_src = xt_d[b].rearrange("(c p) t -> p c t", p=P)
            for c in range(KC):
                nc.sync.dma_start(xt[:, c], xt_src[:, c])

            # ---- v projection: v_all[:, sc, pair, 0:65]   = [v_h0 | ones]
            #                    v_all[:, sc, pair, 65:130] = [v_h1 | ones]
            v_all = p_vall.tile([P, SC, npair, 130], bf16, tag="vall", name="v_all")
            ones_view = v_all.rearrange("p s r (h x) -> p s r h x", h=2)
            nc.gpsimd.memset(ones_view[:, :, :, :, 64:65], 1.0)
            nhalf = (npair + 2) // 3  # groups of <=3 pairs per psum tile
            for tci in range(TC):
                gns = [min(3, npair - 3 * g) for g in range(nhalf)]
                psvs = [ps_m.tile([P, 3 * dpair], f32, tag="m", name="psv") for _ in range(nhalf)]
                for c in range(KC):
                    for g in range(nhalf):
                        nc.tensor.matmul(
                            psvs[g][:, : gns[g] * dpair],
                            lhsT=xt[:, c, tci * P : (tci + 1) * P],
                            rhs=wv_sb[:, c, 3 * g * dpair : (3 * g + gns[g]) * dpair],
                            start=(c == 0),
                            stop=(c == KC - 1),
                        )
                for g in range(nhalf):
                    glo, gn = 3 * g, gns[g]
                    dst = v_all[:, tci, glo : glo + gn, :].rearrange(
                        "p r (h x) -> p r h x", h=2
                    )[:, :, :, 0:64]
                    src = psvs[g][:, : gn * dpair].rearrange("p (r h e) -> p r h e", r=gn, h=2)
                    bias = bv_sb[:, glo : glo + gn, :].rearrange("p r (h e) -> p r h e", h=2)
                    nc.vector.tensor_add(out=dst, in0=src, in1=bias)

            o_allT = p_oall.tile([P, npair, t], bf16, tag="oall", name="o_allT")

            for pr in range(npair):
                # ---- q/k head-pair projections -> qT/kT [128, t] bf16
                qT = p_qk.tile([P, t], bf16, tag="qT", name="qT")
                kT = p_qk.tile([P, t], bf16, tag="kT", name="kT")
                for w_sb, bj, dstT in ((wq_sb, 0, qT), (wk_sb, 1, kT)):
                    psqs = [ps_m.tile([P, TW], f32, tag="m", name="psq") for _ in range(NTH)]
                    for c in range(KC):
                        for th in range(NTH):
                            nc.tensor.matmul(
                                psqs[th][:],
                                lhsT=w_sb[:, pr, c, :],
                                rhs=xt[:, c, th * TW : (th + 1) * TW],
                                start=(c == 0),
                                stop=(c == KC - 1),
                            )
                    for th in range(NTH):
                        nc.vector.tensor_scalar_add(
                            out=dstT[:, th * TW : (th + 1) * TW],
                            in0=psqs[th][:],
                            scalar1=bqk_sb[:, pr, bj : bj + 1],
                        )

                # ---- fused S -> exp -> O pipeline per t-half, with one-stage
                # skew (S(sc+1) emitted before O(sc)) and qk(p+1) dripped in
                for th in range(NTH):
                    es = p_es.tile([P, SC, 2, TW], bf16, tag="es", name="es")
                    psos = [ps_o.tile([65, TW], f32, tag="o", name="pso") for _ in range(2)]
                    for sc in range(SC + 2):
                        if sc < SC:
                            ps = ps_s.tile([P, 2, TW], f32, tag="s", name="ps_s")
                            nc.tensor.matmul(
                                ps[:, 0, :],
                                lhsT=kT[0:64, sc * P : (sc + 1) * P],
                                rhs=qT[0:64, th * TW : (th + 1) * TW],
                                start=True,
                                stop=True,
                            )
                            nc.tensor.matmul(
                                ps[:, 1, :],
                                lhsT=kT[64:128, sc * P : (sc + 1) * P],
                                rhs=qT[64:128, th * TW : (th + 1) * TW],
                                start=True,
                                stop=True,
                                tile_position=None if "notile" in variant else (64, 0),
                            )
                            nc.scalar.activation(
                                out=es[:, sc, :, :], in_=ps[:], func=AF.Exp, scale=scale
                            )
                        if sc >= 2:
                            so = sc - 2
                            for h in range(2):
                                nc.tensor.matmul(
                                    psos[h][:],
                                    lhsT=v_all[:, so, pr, 65 * h : 65 * h + 65],
                                    rhs=es[:, so, h, :],
                                    start=(so == 0),
                                    stop=(so == SC - 1),
                                )
                    # invert the l rows straight out of PSUM (same lane 64),
                    # then DMA the reciprocals down to partition 0 where
                    # partition_broadcast can read them
                    l_sb = p_norm.tile([65, 2, TW], f32, tag="l", name="l_sb")
                    for h in range(2):
                        nc.vector.tensor_copy(out=l_sb[64:65, h, :], in_=psos[h][64:65, :])
                    lg = p_norm.tile([1, 2, TW], f32, tag="lg", name="lg")
                    nc.sync.dma_start(out=lg[0:1, :, :], in_=l_sb[64:65, :, :])
                    lginv = p_norm.tile([1, 2, TW], f32, tag="lginv", name="lginv")
                    if "norecip" in variant:
                        nc.vector.tensor_copy(out=lginv[:], in_=lg[:])
                    else:
                        # custom DVE ops only work at base partition 0 on HW
                        nc.vector.reciprocal_approx_fast(out=lginv[:], in_=lg[:])
                    linv = p_norm.tile([64, 2, TW], f32, tag="linv", name="linv")
                    if "nobcast" in variant:
                        nc.vector.memset(linv[:], 1.0)
                    else:
                        for h in range(2):
                            nc.gpsimd.partition_broadcast(
                                out_ap=linv[:, h, :],
                                in_ap=lginv[0:1, h, :],
                                channels=64,
                            )
                    for h in range(2):
                        if h == 0:
                            nc.vector.tensor_mul(
                                out=o_allT[0:64, pr, th * TW : (th + 1) * TW],
                                in0=psos[h][0:64, :],
                                in1=linv[:, h, :],
                            )
                        else:
                            ot = p_norm.tile([64, TW], bf16, tag="ot", name="ot")
                            nc.vector.tensor_mul(out=ot[:], in0=psos[h][0:64, :], in1=linv[:, h, :])
                            nc.sync.dma_start(
                                out=o_allT[64:128, pr, th * TW : (th + 1) * TW], in_=ot[:]
                            )
            # ---- output projection y = O_all @ Wp + bp
            # last b: ps_m is idle during the final pair's chains (no next-pair
            # qk), so y can start accumulating early pairs there; earlier b's
            # keep y on ps_o so it doesn't contend with the next b's v-proj
            pool_y, tag_y = (ps_m, "m") if b == nb - 1 else (ps_o, "o")
            for tci in range(TC):
                psy = [pool_y.tile([P, D2], f32, tag=tag_y, name="psy") for _ in range(2)]
                for c in range(KC):
                    for j in range(2):
                        nc.tensor.matmul(
                            psy[j][:],
                            lhsT=o_allT[:, c, tci * P : (tci + 1) * P],
                            rhs=wp_sb[:, c, j * D2 : (j + 1) * D2],
                            start=(c == 0),
                            stop=(c == KC - 1),
                        )
                y_sb = p_y.tile([P, d], f32, tag="y", name="y_sb")
                for j in range(2):
                    nc.vector.tensor_add(
                        out=y_sb[:, j * D2 : (j + 1) * D2],
                        in0=psy[j][:],
                        in1=bp_sb[:, j * D2 : (j + 1) * D2],
                    )
                nc.sync.dma_start(out=y_d[b, tci * P : (tci + 1) * P, :], in_=y_sb[:])

    nc.compile()
    return nc


class TileOrExit:
    """Combined TileContext + ExitStack context manager."""

    def __init__(self, nc):
        self.nc = nc
        self.ctx = ExitStack()
        self.tc = tile.TileContext(nc)

    def __enter__(self):
        self.ctx.__enter__()
        self.tc.__enter__()
        return self.tc, self.ctx

    def __exit__(self, *a):
        # close pools before TileContext exits scheduling
        self.ctx.__exit__(*a)
        return self.tc.__exit__(*a)


def prep_inputs(x, Wq, bq, Wk, bk, Wv, bv, Wp, bp, nb, npair):
    """Host-side packing into the DRAM layouts the device kernel expects.

    Returns (shared weight map, list of per-core input maps)."""
    P = 128
    t = x.shape[1]
    d = x.shape[2]
    KC = d // P
    dpair = 2 * HS

    def to_bf(a):
        return np.ascontiguousarray(a).astype(BF16)

    # x^T per batch element
    xt = np.ascontiguousarray(x.transpose(0, 2, 1)).astype(BF16)  # [B, d, t]

    # wq/wk: [P, pair, c, 128] with cols 0:64 = head 2p, 64:128 = head 2p+1
    def pack_qk(W):
        # W: [H, d, HS] -> [pair, 2, KC, P, HS] -> [P, pair, KC, 2*HS]
        w = W.reshape(npair, 2, KC, P, HS)
        w = w.transpose(3, 0, 2, 1, 4).reshape(P, npair, KC, dpair)
        return to_bf(w)

    wq = pack_qk(Wq)
    wk = pack_qk(Wk)
    wv = pack_qk(Wv).transpose(0, 2, 1, 3).reshape(P, KC, npair * dpair)
    wv = np.ascontiguousarray(wv)  # [P, c, pair*128]
    # wp: [P, c, d]
    wp = to_bf(Wp.reshape(KC, P, d).transpose(1, 0, 2))
    # bqk: [P, pair, 2] fp32: partition = pair-stacked head dims
    bqk = np.stack(
        [bq.reshape(npair, dpair), bk.reshape(npair, dpair)], axis=-1
    )  # [pair, 128, 2]
    bqk = np.ascontiguousarray(bqk.transpose(1, 0, 2)).astype(np.float32)  # [P, pair, 2]
    # bv broadcast along t partitions: [P, pair, 128]
    bv_bc = np.broadcast_to(bv.reshape(1, npair, dpair), (P, npair, dpair))
    bv_bc = to_bf(bv_bc)
    # bp broadcast: [P, d] fp32
    bp_bc = np.ascontiguousarray(np.broadcast_to(bp.reshape(1, d), (P, d))).astype(
        np.float32
    )

    weights = {
        "wq": wq,
        "wk": wk,
        "wv": wv,
        "wp": wp,
        "bqk": bqk,
        "bv": bv_bc,
        "bp": bp_bc,
    }
    n_cores = x.shape[0] // nb
    in_maps = []
    for i in range(n_cores):
        m = dict(weights)
        m["xt"] = np.ascontiguousarray(xt[i * nb : (i + 1) * nb])
        in_maps.append(m)
    return in_maps


_NC_CACHE = {}
LAST_RESULT = {}


def kernel(x, Wq, bq, Wk, bk, Wv, bv, Wp, bp, _trace=False):
    x = np.asarray(x, dtype=np.float32)
    Wq, bq = np.asarray(Wq, np.float32), np.asarray(bq, np.float32)
    Wk, bk = np.asarray(Wk, np.float32), np.asarray(bk, np.float32)
    Wv, bv = np.asarray(Wv, np.float32), np.asarray(bv, np.float32)
    Wp, bp = np.asarray(Wp, np.float32), np.asarray(bp, np.float32)

    npair = H // 2
    key = ("full", NB, T_FULL, D_FULL, npair)
    if key not in _NC_CACHE:
        _NC_CACHE[key] = build_mha_nc(NB, T_FULL, D_FULL, npair)
    nc = _NC_CACHE[key]

    in_maps = prep_inputs(x, Wq, bq, Wk, bk, Wv, bv, Wp, bp, NB, npair)
    res = run_bass_kernel_spmd(
        nc, in_maps, core_ids=list(range(N_CORES)), trace=_trace
    )
    LAST_RESULT["exec_time_ns"] = res.exec_time_ns
    LAST_RESULT["res"] = res
    outs = [res.results[i]["y"] for i in range(N_CORES)]
    return np.concatenate(outs, axis=0).astype(np.float32)


# revision 31
# speedup vs baseline: 1.3029x; 1.0032x over previous
"""Multi-head attention (B=16, T=1024, D=768, H=12) on 8 TRN2 NeuronCores.

Strategy: pure data parallelism over the batch — each core computes full MHA
for 2 batch elements. No collectives.

Device kernel design (per core, bf16 compute / fp32 accumulate):
  - Host pre-transposes x to xT[b] = x[b].T ([D, T]) and pre-packs all weights
    in SBUF-ready layouts, cast to bf16.
  - Heads are processed in pairs (2 x HS = 128 = partition width).
  - qT/kT ([128, T], head pair stacked on partitions) come from
    matmul(lhsT=W_pair[dchunk, 128], rhs=xT[dchunk, T]) accumulated over D.
  - S^T[s, t] per head via row-tiled (tile_position) K=64 matmuls packing both
    heads of a pair into the 128-row PE array concurrently.
  - exp via ScalarE activation (scale=1/sqrt(HS) folded in, no max subtraction:
    |S|/8 <= ~3 for this data, exp is safe in fp32->bf16).
  - O^T = v_aug^T @ expS^T with v_aug = [v | ones]: row 64 of the PSUM result
    is the softmax denominator l[t] for free.
  - l -> 1/l via reciprocal_approx_fast (rows of several (h, th) batched on
    partitions via small DMA gathers), broadcast across partitions via a
    DRAM-bounce DMA, normalization as a single DVE multiply per (h, th).
  - y = O_all @ Wp + bp with lhsT = O_all^T (naturally produced above).
"""

import os
from contextlib import ExitStack

import numpy as np
import ml_dtypes

import concourse.bacc as bacc
import concourse.bass as bass
import concourse.mybir as mybir
import concourse.tile as tile
from concourse.bass_utils import run_bass_kernel_spmd

BF16 = ml_dtypes.bfloat16

# Full problem dims
B, T_FULL, D_FULL, H, HS = 16, 1024, 768, 12, 64
N_CORES = 8
NB = B // N_CORES  # batch elements per core


def build_mha_nc(nb, t, d, npair, trn_type="TRN2", variant="full"):
    """Build the Bass program for `nb` batch elements, seq len `t`, model dim
    `d`, `npair` head pairs (each pair = 128 partition lanes)."""
    P = 128
    KC = d // P              # contraction chunks over model dim
    SC = t // P              # s (key position) chunks
    NTH = max(1, t // 512)   # output-column groups for S/O matmuls
    TW = t // NTH            # width of each group (<= 512)
    TC = t // P              # t row chunks for v/y
    D2 = d // 2              # y-proj free-dim split (<= 512 fp32 psum)
    dpair = 2 * HS           # 128
    scale = 1.0 / np.sqrt(HS)

    f32 = mybir.dt.float32
    bf16 = mybir.dt.bfloat16
    AF = mybir.ActivationFunctionType

    nc = bacc.Bacc(trn_type, target_bir_lowering=False, debug=False)

    xt_d = nc.dram_tensor("xt", [nb, d, t], bf16, kind="ExternalInput").ap()
    wq_d = nc.dram_tensor("wq", [P, npair, KC, dpair], bf16, kind="ExternalInput").ap()
    wk_d = nc.dram_tensor("wk", [P, npair, KC, dpair], bf16, kind="ExternalInput").ap()
    wv_d = nc.dram_tensor("wv", [P, KC, npair * dpair], bf16, kind="ExternalInput").ap()
    wp_d = nc.dram_tensor("wp", [P, KC, d], bf16, kind="ExternalInput").ap()
    bqk_d = nc.dram_tensor("bqk", [P, npair, 2], f32, kind="ExternalInput").ap()
    bv_d = nc.dram_tensor("bv", [P, npair, dpair], bf16, kind="ExternalInput").ap()
    bp_d = nc.dram_tensor("bp", [P, d], f32, kind="ExternalInput").ap()
    y_d = nc.dram_tensor("y", [nb, t, d], f32, kind="ExternalOutput").ap()

    with TileOrExit(nc) as (tc, ctx):
        # ---- persistent weights (one bufs=1 pool; each tag allocated once) ----
        p_w = ctx.enter_context(tc.tile_pool(name="p_w", bufs=1))
        wq_sb = p_w.tile([P, npair, KC, dpair], bf16, tag="wq", name="wq_sb")
        wk_sb = p_w.tile([P, npair, KC, dpair], bf16, tag="wk", name="wk_sb")
        wv_sb = p_w.tile([P, KC, npair * dpair], bf16, tag="wv", name="wv_sb")
        wp_sb = p_w.tile([P, KC, d], bf16, tag="wp", name="wp_sb")
        bqk_sb = p_w.tile([P, npair, 2], f32, tag="bqk", name="bqk_sb")
        bv_sb = p_w.tile([P, npair, dpair], bf16, tag="bv", name="bv_sb")
        bp_sb = p_w.tile([P, d], f32, tag="bp", name="bp_sb")
        # weight loads ride the gpsimd DMA queue so the sync queue is free for
        # xt (first compute dependency); split by chunk for fine-grained deps
        for c in range(KC):
            nc.gpsimd.dma_start(wv_sb[:, c], wv_d[:, c])
        nc.gpsimd.dma_start(bv_sb[:], bv_d)
        for pr in range(npair):
            nc.gpsimd.dma_start(wq_sb[:, pr], wq_d[:, pr])
            nc.gpsimd.dma_start(wk_sb[:, pr], wk_d[:, pr])
        nc.gpsimd.dma_start(bqk_sb[:], bqk_d)
        nc.gpsimd.dma_start(wp_sb[:], wp_d)
        nc.gpsimd.dma_start(bp_sb[:], bp_d)

        # ---- pools ----
        p_xt = ctx.enter_context(tc.tile_pool(name="p_xt", bufs=2))
        p_vall = ctx.enter_context(tc.tile_pool(name="p_vall", bufs=2))
        p_qk = ctx.enter_context(tc.tile_pool(name="p_qk", bufs=4))
        p_es = ctx.enter_context(tc.tile_pool(name="p_es", bufs=3))
        p_oall = ctx.enter_context(tc.tile_pool(name="p_oall", bufs=1))
        p_norm = ctx.enter_context(tc.tile_pool(name="p_norm", bufs=2))
        p_y = ctx.enter_context(tc.tile_pool(name="p_y", bufs=2))
        p_dram = ctx.enter_context(tc.tile_pool(name="p_dram", bufs=2, space="DRAM"))
        ps_s = ctx.enter_context(tc.tile_pool(name="ps_s", bufs=2, space="PSUM"))
        ps_o = ctx.enter_context(tc.tile_pool(name="ps_o", bufs=2, space="PSUM"))
        ps_m = ctx.enter_context(tc.tile_pool(name="ps_m", bufs=2, space="PSUM"))

        # HAM warm-up: a burst of dummy matmuls during the initial DMA wait
        # so the PE clock is at 2.4 GHz when real work arrives
        warm = p_norm.tile([P, TW], bf16, tag="warm", name="warm")
        nc.vector.memset(warm[:], 0.0)
        wps = ps_m.tile([P, TW], f32, tag="m", name="wps")
        for i in range(24):
            nc.tensor.matmul(
                wps[:], lhsT=warm[:, 0:P], rhs=warm[:], start=(i == 0), stop=(i == 23)
            )

        for b in range(nb):
            xt = p_xt.tile([P, KC, t], bf16, tag="xt", name="xt_sb")
            xt_src = xt_d[b].rearrange("(c p) t -> p c t", p=P)
            for c in range(KC):
                nc.sync.dma_start(xt[:, c], xt_src[:, c])

            # ---- v projection: v_all[:, sc, pair, 0:65]   = [v_h0 | ones]
            #                    v_all[:, sc, pair, 65:130] = [v_h1 | ones]
            v_all = p_vall.tile([P, SC, npair, 130], bf16, tag="vall", name="v_all")
            ones_view = v_all.rearrange("p s r (h x) -> p s r h x", h=2)
            nc.gpsimd.memset(ones_view[:, :, :, :, 64:65], 1.0)
            nhalf = (npair + 2) // 3  # groups of <=3 pairs per psum tile
            for tci in range(TC):
                gns = [min(3, npair - 3 * g) for g in range(nhalf)]
                psvs = [ps_m.tile([P, 3 * dpair], f32, tag="m", name="psv") for _ in range(nhalf)]
                for c in range(KC):
                    for g in range(nhalf):
                        nc.tensor.matmul(
                            psvs[g][:, : gns[g] * dpair],
                            lhsT=xt[:, c, tci * P : (tci + 1) * P],
                            rhs=wv_sb[:, c, 3 * g * dpair : (3 * g + gns[g]) * dpair],
                            start=(c == 0),
                            stop=(c == KC - 1),
                        )
                for g in range(nhalf):
                    glo, gn = 3 * g, gns[g]
                    dst = v_all[:, tci, glo : glo + gn, :].rearrange(
                        "p r (h x) -> p r h x", h=2
                    )[:, :, :, 0:64]
                    src = psvs[g][:, : gn * dpair].rearrange("p (r h e) -> p r h e", r=gn, h=2)
                    bias = bv_sb[:, glo : glo + gn, :].rearrange("p r (h e) -> p r h e", h=2)
                    nc.vector.tensor_add(out=dst, in0=src, in1=bias)

            o_allT = p_oall.tile([P, npair, t], bf16, tag="oall", name="o_allT")

            for pr in range(npair):
                # ---- q/k head-pair projections -> qT/kT [128, t] bf16
                qT = p_qk.tile([P, t], bf16, tag="qT", name="qT")
                kT = p_qk.tile([P, t], bf16, tag="kT", name="kT")
                for w_sb, bj, dstT in ((wq_sb, 0, qT), (wk_sb, 1, kT)):
                    psqs = [ps_m.tile([P, TW], f32, tag="m", name="psq") for _ in range(NTH)]
                    for c in range(KC):
                        for th in range(NTH):
                            nc.tensor.matmul(
                                psqs[th][:],
                                lhsT=w_sb[:, pr, c, :],
                                rhs=xt[:, c, th * TW : (th + 1) * TW],
                                start=(c == 0),
                                stop=(c == KC - 1),
                            )
                    for th in range(NTH):
                        nc.vector.tensor_scalar_add(
                            out=dstT[:, th * TW : (th + 1) * TW],
                            in0=psqs[th][:],
                            scalar1=bqk_sb[:, pr, bj : bj + 1],
                        )

                # ---- fused S -> exp -> O pipeline per t-half, with one-stage
                # skew (S(sc+1) emitted before O(sc)) and qk(p+1) dripped in
                for th in range(NTH):
                    es = p_es.tile([P, SC, 2, TW], bf16, tag="es", name="es")
                    psos = [ps_o.tile([65, TW], f32, tag="o", name="pso") for _ in range(2)]
                    for sc in range(SC + 2):
                        if sc < SC:
                            ps = ps_s.tile([P, 2, TW], f32, tag="s", name="ps_s")
                            nc.tensor.matmul(
                                ps[:, 0, :],
                                lhsT=kT[0:64, sc * P : (sc + 1) * P],
                                rhs=qT[0:64, th * TW : (th + 1) * TW],
                                start=True,
                                stop=True,
                            )
                            nc.tensor.matmul(
                                ps[:, 1, :],
                                lhsT=kT[64:128, sc * P : (sc + 1) * P],
                                rhs=qT[64:128, th * TW : (th + 1) * TW],
                                start=True,
                                stop=True,
                                tile_position=None if "notile" in variant else (64, 0),
                            )
                            nc.scalar.activation(
                                out=es[:, sc, :, :], in_=ps[:], func=AF.Exp, scale=scale
                            )
                        if sc >= 2:
                            so = sc - 2
                            for h in range(2):
                                nc.tensor.matmul(
                                    psos[h][:],
                                    lhsT=v_all[:, so, pr, 65 * h : 65 * h + 65],
                                    rhs=es[:, so, h, :],
                                    start=(so == 0),
                                    stop=(so == SC - 1),
                                )
                    # invert the l rows straight out of PSUM (same lane 64),
                    # then DMA the reciprocals down to partition 0 where
                    # partition_broadcast can read them
                    l_sb = p_norm.tile([65, 2, TW], f32, tag="l", name="l_sb")
                    for h in range(2):
                        nc.vector.tensor_copy(out=l_sb[64:65, h, :], in_=psos[h][64:65, :])
                    lg = p_norm.tile([1, 2, TW], f32, tag="lg", name="lg")
                    nc.sync.dma_start(out=lg[0:1, :, :], in_=l_sb[64:65, :, :])
                    lginv = p_norm.tile([1, 2, TW], f32, tag="lginv", name="lginv")
                    if "norecip" in variant:
                        nc.vector.tensor_copy(out=lginv[:], in_=lg[:])
                    else:
                        # custom DVE ops only work at base partition 0 on HW
                        nc.vector.reciprocal_approx_fast(out=lginv[:], in_=lg[:])
                    linv = p_norm.tile([64, 2, TW], f32, tag="linv", name="linv")
                    if "nobcast" in variant:
                        nc.vector.memset(linv[:], 1.0)
                    else:
                        for h in range(2):
                            nc.gpsimd.partition_broadcast(
                                out_ap=linv[:, h, :],
                                in_ap=lginv[0:1, h, :],
                                channels=64,
                            )
                    for h in range(2):
                        if h == 0:
                            nc.vector.tensor_mul(
                                out=o_allT[0:64, pr, th * TW : (th + 1) * TW],
                                in0=psos[h][0:64, :],
                                in1=linv[:, h, :],
                            )
                        else:
                            ot = p_norm.tile([64, TW], bf16, tag="ot", name="ot")
                            nc.vector.tensor_mul(out=ot[:], in0=psos[h][0:64, :], in1=linv[:, h, :])
                            nc.sync.dma_start(
                                out=o_allT[64:128, pr, th * TW : (th + 1) * TW], in_=ot[:]
                            )
            # ---- output projection y = O_all @ Wp + bp
            # last b: ps_m is idle during the final pair's chains (no next-pair
            # qk), so y can start accumulating early pairs there; earlier b's
            # keep y on ps_o so it doesn't contend with the next b's v-proj
            pool_y, tag_y = (ps_m, "m") if b == nb - 1 else (ps_o, "o")
            for tci in range(TC):
                psy = [pool_y.tile([P, D2], f32, tag=tag_y, name="psy") for _ in range(2)]
                for c in range(KC):
                    for j in range(2):
                        nc.tensor.matmul(
                            psy[j][:],
                            lhsT=o_allT[:, c, tci * P : (tci + 1) * P],
                            rhs=wp_sb[:, c, j * D2 : (j + 1) * D2],
                            start=(c == 0),
                            stop=(c == KC - 1),
                        )
                y_sb = p_y.tile([P, d], f32, tag="y", name="y_sb")
                for j in range(2):
                    nc.vector.tensor_add(
                        out=y_sb[:, j * D2 : (j + 1) * D2],
                        in0=psy[j][:],
                        in1=bp_sb[:, j * D2 : (j + 1) * D2],
                    )
                nc.sync.dma_start(out=y_d[b, tci * P : (tci + 1) * P, :], in_=y_sb[:])

    nc.compile()
    return nc


class TileOrExit:
    """Combined TileContext + ExitStack context manager."""

    def __init__(self, nc):
        self.nc = nc
        self.ctx = ExitStack()
        self.tc = tile.TileContext(nc)

    def __enter__(self):
        self.ctx.__enter__()
        self.tc.__enter__()
        return self.tc, self.ctx

    def __exit__(self, *a):
        # close pools before TileContext exits scheduling
        self.ctx.__exit__(*a)
        return self.tc.__exit__(*a)


def prep_inputs(x, Wq, bq, Wk, bk, Wv, bv, Wp, bp, nb, npair):
    """Host-side packing into the DRAM layouts the device kernel expects.

    Returns (shared weight map, list of per-core input maps)."""
    P = 128
    t = x.shape[1]
    d = x.shape[2]
    KC = d // P
    dpair = 2 * HS

    def to_bf(a):
        return np.ascontiguousarray(a).astype(BF16)

    # x^T per batch element
    xt = np.ascontiguousarray(x.transpose(0, 2, 1)).astype(BF16)  # [B, d, t]

    # wq/wk: [P, pair, c, 128] with cols 0:64 = head 2p, 64:128 = head 2p+1
    def pack_qk(W):
        # W: [H, d, HS] -> [pair, 2, KC, P, HS] -> [P, pair, KC, 2*HS]
        w = W.reshape(npair, 2, KC, P, HS)
        w = w.transpose(3, 0, 2, 1, 4).reshape(P, npair, KC, dpair)
        return to_bf(w)

    wq = pack_qk(Wq)
    wk = pack_qk(Wk)
    wv = pack_qk(Wv).transpose(0, 2, 1, 3).reshape(P, KC, npair * dpair)
    wv = np.ascontiguousarray(wv)  # [P, c, pair*128]
    # wp: [P, c, d]
    wp = to_bf(Wp.reshape(KC, P, d).transpose(1, 0, 2))
    # bqk: [P, pair, 2] fp32: partition = pair-stacked head dims
    bqk = np.stack(
        [bq.reshape(npair, dpair), bk.reshape(npair, dpair)], axis=-1
    )  # [pair, 128, 2]
    bqk = np.ascontiguousarray(bqk.transpose(1, 0, 2)).astype(np.float32)  # [P, pair, 2]
    # bv broadcast along t partitions: [P, pair, 128]
    bv_bc = np.broadcast_to(bv.reshape(1, npair, dpair), (P, npair, dpair))
    bv_bc = to_bf(bv_bc)
    # bp broadcast: [P, d] fp32
    bp_bc = np.ascontiguousarray(np.broadcast_to(bp.reshape(1, d), (P, d))).astype(
        np.float32
    )

    weights = {
        "wq": wq,
        "wk": wk,
        "wv": wv,
        "wp": wp,
        "bqk": bqk,
        "bv": bv_bc,
        "bp": bp_bc,
    }
    n_cores = x.shape[0] // nb
    in_maps = []
    for i in range(n_cores):
        m = dict(weights)
        m["xt"] = np.ascontiguousarray(xt[i * nb : (i + 1) * nb])
        in_maps.append(m)
    return in_maps


_NC_CACHE = {}
LAST_RESULT = {}


def kernel(x, Wq, bq, Wk, bk, Wv, bv, Wp, bp, _trace=False):
    x = np.asarray(x, dtype=np.float32)
    Wq, bq = np.asarray(Wq, np.float32), np.asarray(bq, np.float32)
    Wk, bk = np.asarray(Wk, np.float32), np.asarray(bk, np.float32)
    Wv, bv = np.asarray(Wv, np.float32), np.asarray(bv, np.float32)
    Wp, bp = np.asarray(Wp, np.float32), np.asarray(bp, np.float32)

    npair = H // 2
    key = ("full", NB, T_FULL, D_FULL, npair)
    if key not in _NC_CACHE:
        _NC_CACHE[key] = build_mha_nc(NB, T_FULL, D_FULL, npair)
    nc = _NC_CACHE[key]

    in_maps = prep_inputs(x, Wq, bq, Wk, bk, Wv, bv, Wp, bp, NB, npair)
    res = run_bass_kernel_spmd(
        nc, in_maps, core_ids=list(range(N_CORES)), trace=_trace
    )
    LAST_RESULT["exec_time_ns"] = res.exec_time_ns
    LAST_RESULT["res"] = res
    outs = [res.results[i]["y"] for i in range(N_CORES)]
    return np.concatenate(outs, axis=0).astype(np.float32)
